# revision 26
# baseline (speedup 1.0000x reference)
"""DeepseekV3 decoder layer (MLA + SwiGLU MLP), T=2048 prefill, fp32 I/O.

Sharding: sequence-parallel striped — core c owns token rows c::8 (256 rows),
so all 8 cores run one identical SPMD program with balanced causal work; only
input data differs per core. The KV latent path (all 2048 tokens) is
replicated on every core; outputs are disjoint row sets concatenated on host.

Per core: row-major activations (per-token norm scales are per-partition),
bf16 matmul operands with fp32 PSUM accumulation, LN weights folded into
adjacent GEMMs on host, RoPE via host cos/sin tables, softmax without
max-subtraction (scores are O(30) max), denominator via ones-matmul,
causal masking by static tile skipping + mask multiply. The q_b/absorb
path runs lazily per head inside the attention loop to bound SBUF.
"""

import numpy as np
import ml_dtypes

bfloat16 = ml_dtypes.bfloat16

T = 2048
H = 2048
NH = 16
QLR = 1536
KVLR = 512
DN = 128
DR = 64
DV = 128
INTER = 10944
NCORES = 8
RPC = T // NCORES
NQT = RPC // 128
NTT = T // 128
NFC = H // 128
NRC = QLR // 128
NKV = KVLR // 128
NIT = 86
IPAD = NIT * 128
EPS = 1e-6
SCALE = (DN + DR) ** -0.5
THETA = 10000.0
QH = DN + DR               # 192 per-head q dim

_CACHE = {}


def _build_module():
    import os
    MAXPH = int(os.environ.get("KERNEL_MAXPH", "9"))
    import concourse.bass as bass
    import concourse.tile as tile
    from concourse import bacc, mybir

    f32 = mybir.dt.float32
    bf16 = mybir.dt.bfloat16
    f8e4 = mybir.dt.float8e4
    DROW = mybir.MatmulPerfMode.DoubleRow
    AF = mybir.ActivationFunctionType
    ALU = mybir.AluOpType

    nc = bacc.Bacc("TRN2", target_bir_lowering=False, debug=False,
                   enable_asserts=False, num_devices=NCORES)

    def inp(name, shape, dt):
        return nc.dram_tensor(name, list(shape), dt, kind="ExternalInput").ap()

    # per-core inputs
    x_rows = inp("x_rows", [NQT, 128, H], f32)
    xTc = inp("xTc", [NFC, 128, RPC], bf16)
    cosqT = inp("cosqT", [64, RPC], f32)
    sinqT = inp("sinqT", [64, RPC], f32)
    masks = inp("masks", [NTT, 128, RPC], bf16)
    permT = inp("permT", [64, 64], bf16)
    # replicated inputs
    xstat = inp("xstat", [NTT, 128, H], bf16)
    xT_blk = inp("xT_blk", [NTT, 128, NFC, 128], bf16)
    qa_blk = inp("qa_blk", [NFC, 128, QLR], bf16)
    qb_blk = inp("qb_blk", [NH, NRC, 128, QH], bf16)
    kva_blk = inp("kva_blk", [NFC, 128, KVLR + DR], bf16)
    wuk = inp("wuk", [NH, 128, NKV, 128], bf16)
    wuv = inp("wuv", [NH, 128, NKV, DV], bf16)
    ow_blk = inp("ow_blk", [NH, 128, H], bf16)
    gu_blk = inp("gu_blk", [2, NIT, 128, NFC, 128], bf16)
    dw_blk = inp("dw_blk", [NIT, 128, H], bf16)
    cosk = inp("cosk", [128, NTT, DR // 2], f32)
    sink = inp("sink", [128, NTT, DR // 2], f32)
    eye = inp("eye", [128, 128], bf16)
    ones = inp("ones", [128, 1], bf16)

    out_rows = nc.dram_tensor("out_rows", [NQT, 128, H], f32,
                              kind="ExternalOutput").ap()

    from contextlib import ExitStack
    with tile.TileContext(nc) as tc, ExitStack() as ctx:
        persist = ctx.enter_context(tc.tile_pool(name="persist", bufs=1))

        def pt(shape, dt, tag):
            return persist.tile(list(shape), dt, tag=tag, name=tag)

        eps_sb = pt([128, 1], f32, "eps")
        nc.vector.memset(eps_sb[:], EPS)
        eye_sb = pt([128, 128], bf16, "eye")
        nc.sync.dma_start(out=eye_sb[:], in_=eye[:])
        ones_sb = pt([128, 1], bf16, "ones")
        nc.sync.dma_start(out=ones_sb[:], in_=ones[:])
        x_rows_sb = pt([128, NQT, H], f32, "x_rows")
        for qt in range(NQT):
            nc.sync.dma_start(out=x_rows_sb[:, qt, :], in_=x_rows[qt])

        rstd_all = pt([128, NTT], f32, "rstd_all")
        s_ck = pt([128, NTT], f32, "s_ck")
        c_hat = pt([128, NTT, KVLR], bf16, "c_hat")
        kT_lat = pt([128, NKV, T], bf16, "kT_lat")
        kT_rope = pt([64, T], bf16, "kT_rope")
        qcT = pt([128, NRC, RPC], bf16, "qcT")
        o_vT = pt([128, NH, RPC], bf16, "o_vT")
        hnT = pt([128, NFC, RPC], bf16, "hnT")
        act_all = pt([128, NIT, RPC], bf16, "act_all")

        # =================== phase 0: stats + kv path ===================
        with tc.tile_pool(name="p0", bufs=3) as p0, \
             tc.tile_pool(name="p0w", bufs=NFC) as p0w, \
             tc.tile_pool(name="p0s", bufs=1) as p0s, \
             tc.tile_pool(name="p0d", bufs=2) as p0d, \
             tc.tile_pool(name="p0ps", bufs=2, space="PSUM") as p0ps, \
             tc.tile_pool(name="p0tp", bufs=2, space="PSUM") as p0tp:
            cosk_sb = p0s.tile([128, NTT, DR // 2], f32, name="cosk_sb")
            nc.sync.dma_start(out=cosk_sb[:], in_=cosk[:])
            sink_sb = p0s.tile([128, NTT, DR // 2], f32, name="sink_sb")
            nc.sync.dma_start(out=sink_sb[:], in_=sink[:])
            ssq_all = p0s.tile([128, NTT], f32, name="ssq_all")
            ssq_kv = p0s.tile([128, NTT], f32, name="ssq_kv")
            c_raw = p0s.tile([128, NTT, KVLR + DR], bf16, name="c_raw")
            for tt in range(NTT):
                xs = p0.tile([128, H], bf16, tag="xs", name="xs")
                nc.sync.dma_start(out=xs[:], in_=xstat[tt])
                scrap = p0d.tile([128, H], bf16, tag="scrap", name="scrap")
                nc.vector.scalar_tensor_tensor(
                    scrap[:], xs[:], 1.0, xs[:], ALU.bypass, ALU.mult,
                    accum_out=ssq_all[:, tt:tt + 1])
            nc.scalar.activation(rstd_all[:], ssq_all[:], AF.Ln,
                                 bias=eps_sb[:], scale=1.0 / H)
            nc.scalar.activation(rstd_all[:], rstd_all[:], AF.Exp, scale=-0.5)

            kvw = []
            for fc in range(NFC):
                w = p0w.tile([128, KVLR + DR], bf16, tag="kvw", name="kvw")
                nc.sync.dma_start(out=w[:], in_=kva_blk[fc])
                kvw.append(w)
            for tt in range(NTT):
                xt = p0.tile([128, NFC, 128], bf16, tag="xt", name="xt")
                nc.scalar.dma_start(out=xt[:], in_=xT_blk[tt])
                ps = p0ps.tile([128, KVLR + DR], f32, tag="kvps", name="kvps")
                for fc in range(NFC):
                    nc.tensor.matmul(ps[:, 0:512], xt[:, fc, :],
                                     kvw[fc][:, 0:512],
                                     start=(fc == 0), stop=(fc == NFC - 1))
                    nc.tensor.matmul(ps[:, 512:576], xt[:, fc, :],
                                     kvw[fc][:, 512:576],
                                     start=(fc == 0), stop=(fc == NFC - 1))
                scr2 = p0d.tile([128, KVLR], bf16, tag="scr2", name="scr2")
                nc.scalar.activation(scr2[:], ps[:, 0:512], AF.Square,
                                     accum_out=ssq_kv[:, tt:tt + 1])
                nc.vector.tensor_copy(c_raw[:, tt, :], ps[:])
            t1 = p0s.tile([128, NTT], f32, name="t1")
            nc.vector.tensor_mul(t1[:], rstd_all[:], rstd_all[:])
            nc.vector.tensor_mul(t1[:], t1[:], ssq_kv[:])
            nc.scalar.activation(t1[:], t1[:], AF.Ln, bias=eps_sb[:],
                                 scale=1.0 / KVLR)
            nc.scalar.activation(t1[:], t1[:], AF.Exp, scale=-0.5)
            nc.vector.tensor_mul(s_ck[:], rstd_all[:], t1[:])
            for tt in range(NTT):
                nc.vector.tensor_scalar_mul(c_hat[:, tt, :], c_raw[:, tt, 0:512],
                                            s_ck[:, tt:tt + 1])
            kr = p0s.tile([128, NTT, DR], bf16, name="kr")
            krf = p0s.tile([128, NTT, DR], bf16, name="krf")
            for tt in range(NTT):
                nc.vector.tensor_scalar_mul(kr[:, tt, :], c_raw[:, tt, 512:576],
                                            rstd_all[:, tt:tt + 1])
            x1 = kr[:, :, 0:DR:2]
            x2 = kr[:, :, 1:DR:2]
            ta = p0s.tile([128, NTT, DR // 2], f32, name="ta")
            tb = p0s.tile([128, NTT, DR // 2], f32, name="tb")
            nc.vector.tensor_mul(ta[:], x1, cosk_sb[:])
            nc.vector.tensor_mul(tb[:], x2, sink_sb[:])
            nc.vector.tensor_sub(krf[:, :, 0:DR:2], ta[:], tb[:])
            nc.vector.tensor_mul(ta[:], x2, cosk_sb[:])
            nc.vector.tensor_mul(tb[:], x1, sink_sb[:])
            nc.vector.tensor_add(krf[:, :, 1:DR:2], ta[:], tb[:])
            for tt in range(NTT):
                for rc in range(NKV):
                    tp = p0tp.tile([128, 128], bf16, tag="tp", name="tp")
                    nc.tensor.transpose(tp[:],
                                        c_hat[:, tt, rc * 128:(rc + 1) * 128],
                                        eye_sb[:])
                    nc.any.tensor_copy(kT_lat[:, rc, tt * 128:(tt + 1) * 128],
                                       tp[:])
                tp = p0tp.tile([128, 128], bf16, tag="tp", name="tp")
                nc.tensor.transpose(tp[0:64, :], krf[:, tt, :], eye_sb[:])
                nc.any.tensor_copy(kT_rope[:, tt * 128:(tt + 1) * 128],
                                   tp[0:64, :])


        # =================== phase 1: q_a -> qcT ===================
        if MAXPH >= 1:
            with tc.tile_pool(name="p1", bufs=3) as p1, \
                 tc.tile_pool(name="p1s", bufs=1) as p1s, \
                 tc.tile_pool(name="p1d", bufs=2) as p1d, \
                 tc.tile_pool(name="p1ps", bufs=2, space="PSUM") as p1ps, \
                 tc.tile_pool(name="p1tp", bufs=2, space="PSUM") as p1tp:
                rstd_rows = p1s.tile([128, NQT], f32, name="rstd_rows")
                ssq_r = p1s.tile([128, NQT], f32, name="ssq_r")
                for qt in range(NQT):
                    scrap = p1d.tile([128, H], bf16, tag="scrapq", name="scrapq")
                    nc.vector.scalar_tensor_tensor(
                        scrap[:], x_rows_sb[:, qt, :], 1.0, x_rows_sb[:, qt, :],
                        ALU.bypass, ALU.mult, accum_out=ssq_r[:, qt:qt + 1])
                nc.scalar.activation(rstd_rows[:], ssq_r[:], AF.Ln,
                                     bias=eps_sb[:], scale=1.0 / H)
                nc.scalar.activation(rstd_rows[:], rstd_rows[:], AF.Exp, scale=-0.5)

                xTc_sb = p1s.tile([128, NFC, RPC], bf16, name="xTc_sb")
                for fc in range(NFC):
                    nc.sync.dma_start(out=xTc_sb[:, fc, :], in_=xTc[fc])
                qa_ps = [p1ps.tile([128, QLR], f32, tag="mm", name="mm")
                         for _ in range(NQT)]
                for fc in range(NFC):
                    qaw = p1.tile([128, QLR], bf16, tag="qaw", name="qaw")
                    nc.sync.dma_start(out=qaw[:], in_=qa_blk[fc])
                    for qt in range(NQT):
                        for nn in range(QLR // 512):
                            nc.tensor.matmul(
                                qa_ps[qt][:, nn * 512:(nn + 1) * 512],
                                xTc_sb[:, fc, qt * 128:(qt + 1) * 128],
                                qaw[:, nn * 512:(nn + 1) * 512],
                                start=(fc == 0), stop=(fc == NFC - 1))
                qc = p1s.tile([128, NQT, QLR], bf16, name="qc")
                ssq_q = p1s.tile([128, NQT], f32, name="ssq_q")
                for qt in range(NQT):
                    scr = p1d.tile([128, QLR], bf16, tag="scrq2", name="scrq2")
                    nc.scalar.activation(scr[:], qa_ps[qt][:], AF.Square,
                                         accum_out=ssq_q[:, qt:qt + 1])
                sq = p1s.tile([128, NQT], f32, name="sq")
                nc.vector.tensor_mul(sq[:], rstd_rows[:], rstd_rows[:])
                nc.vector.tensor_mul(sq[:], sq[:], ssq_q[:])
                nc.scalar.activation(sq[:], sq[:], AF.Ln, bias=eps_sb[:],
                                     scale=1.0 / QLR)
                nc.scalar.activation(sq[:], sq[:], AF.Exp, scale=-0.5)
                nc.vector.tensor_mul(sq[:], rstd_rows[:], sq[:])
                for qt in range(NQT):
                    nc.vector.tensor_scalar_mul(qc[:, qt, :], qa_ps[qt][:],
                                                sq[:, qt:qt + 1])
                for qt in range(NQT):
                    for rc in range(NRC):
                        tp = p1tp.tile([128, 128], bf16, tag="tp", name="tp")
                        nc.tensor.transpose(tp[:], qc[:, qt, rc * 128:(rc + 1) * 128],
                                            eye_sb[:])
                        nc.any.tensor_copy(qcT[:, rc, qt * 128:(qt + 1) * 128],
                                           tp[:])


        # ============ phase 2: head-pair q_b + attention ============
        # Heads processed in pairs: the kT_lat/kT_rope/c_hat stationaries are
        # shared across heads, so a 3D moving AP [128, 2, N] doubles the free
        # dim per LDWEIGHTS and halves the LDW count (the prior bottleneck).
        if MAXPH >= 2:
            with tc.tile_pool(name="p2", bufs=4) as p2, \
                 tc.tile_pool(name="p2s", bufs=1) as p2s, \
                 tc.tile_pool(name="p2d", bufs=2) as p2d, \
                 tc.tile_pool(name="p2e", bufs=4) as p2e, \
                 tc.tile_pool(name="p2r", bufs=2) as p2r, \
                 tc.tile_pool(name="pQ", bufs=1, space="PSUM") as pQ, \
                 tc.tile_pool(name="psp", bufs=2, space="PSUM") as psp, \
                 tc.tile_pool(name="pO", bufs=4, space="PSUM") as pO, \
                 tc.tile_pool(name="pD", bufs=1, space="PSUM") as pD:
                masks_sb = p2s.tile([128, NTT, RPC], bf16, name="masks_sb")
                for kt in range(NTT):
                    nc.sync.dma_start(out=masks_sb[:, kt, :], in_=masks[kt])
                cosqT_sb = p2s.tile([64, RPC], f32, name="cosqT_sb")
                nc.sync.dma_start(out=cosqT_sb[:], in_=cosqT[:])
                sinqT_sb = p2s.tile([64, RPC], f32, name="sinqT_sb")
                nc.sync.dma_start(out=sinqT_sb[:], in_=sinqT[:])
                permT_sb = p2s.tile([64, 64], bf16, name="permT_sb")
                nc.sync.dma_start(out=permT_sb[:], in_=permT[:])

                for hp in range(NH // 2):
                    qT2 = p2d.tile([128, NKV, 2, RPC], bf16, tag="qT2",
                                   name="qT2")
                    qrT2 = p2d.tile([64, 2, RPC], bf16, tag="qrT2", name="qrT2")
                    wuv_hh = []
                    for hh in range(2):
                        h = 2 * hp + hh
                        qbw = p2.tile([128, NRC, QH], bf16, tag="qbw",
                                      name="qbw")
                        for rc in range(NRC):
                            nc.sync.dma_start(out=qbw[:, rc, :],
                                              in_=qb_blk[h, rc])
                        wuk_h = p2.tile([128, NKV, 128], bf16, tag="wuk_h",
                                        name="wuk_h")
                        nc.sync.dma_start(out=wuk_h[:], in_=wuk[h])
                        wuv_h = p2.tile([128, NKV, DV], bf16, tag="wuv_h",
                                        name="wuv_h")
                        nc.sync.dma_start(out=wuv_h[:], in_=wuv[h])
                        wuv_hh.append(wuv_h)

                        # q_b transposed: nope [dn, tok] + rope [dr, tok]
                        # share one PSUM bank; only the first MM carries start
                        qnr_ps = pQ.tile([128, 512], f32, tag="qnr", name="qnr")
                        for rc in range(NRC):
                            nc.tensor.matmul(
                                qnr_ps[:, 0:RPC], qbw[:, rc, 0:DN],
                                qcT[:, rc, :],
                                start=(rc == 0), stop=(rc == NRC - 1),
                                skip_group_check=True)
                            nc.tensor.matmul(
                                qnr_ps[0:DR, RPC:2 * RPC], qbw[:, rc, DN:QH],
                                qcT[:, rc, :],
                                start=False, stop=(rc == NRC - 1),
                                skip_group_check=True)
                        qnT = p2d.tile([128, RPC], bf16, tag="qnT", name="qnT")
                        nc.vector.tensor_copy(qnT[:], qnr_ps[:, 0:RPC])
                        # rope via signed pair-swap perm matmul: C*x + P(S*x)
                        tS = p2d.tile([64, RPC], bf16, tag="tS", name="tS")
                        nc.vector.tensor_mul(tS[:], qnr_ps[0:DR, RPC:2 * RPC],
                                             sinqT_sb[:])
                        tC = p2d.tile([64, RPC], f32, tag="tC", name="tC")
                        nc.vector.tensor_mul(tC[:], qnr_ps[0:DR, RPC:2 * RPC],
                                             cosqT_sb[:])
                        pr_ps = psp.tile([128, 2 * RPC], f32, tag="sp",
                                         name="prps")
                        nc.tensor.matmul(pr_ps[0:DR, 0:RPC], permT_sb[:], tS[:],
                                         start=True, stop=True)
                        nc.vector.tensor_add(qrT2[:, hh, :], tC[:],
                                             pr_ps[0:DR, 0:RPC])
                        # absorb w_uk into q
                        for rc in range(NKV):
                            lp = psp.tile([128, 2 * RPC], f32, tag="sp",
                                          name="lp")
                            nc.tensor.matmul(lp[:, 0:RPC], wuk_h[:, rc, :],
                                             qnT[:], start=True, stop=True)
                            nc.vector.tensor_copy(qT2[:, rc, hh, :],
                                                  lp[:, 0:RPC])

                    olT_ps = [pO.tile([128, 2, RPC], f32, tag="olT", name="olT")
                              for _ in range(NKV)]
                    den_ps = pD.tile([1, 2, RPC], f32, tag="den", name="den")
                    for kt in range(NTT):
                        q0 = 16 * kt
                        sp3 = psp.tile([128, 2, RPC], f32, tag="sp", name="sp")
                        for dc in range(NKV):
                            nc.tensor.matmul(
                                sp3[:, :, q0:],
                                kT_lat[:, dc, kt * 128:(kt + 1) * 128],
                                qT2[:, dc, :, q0:],
                                start=(dc == 0), stop=False,
                                skip_group_check=True)
                        nc.tensor.matmul(
                            sp3[:, :, q0:],
                            kT_rope[:, kt * 128:(kt + 1) * 128],
                            qrT2[:, :, q0:],
                            start=False, stop=True, skip_group_check=True)
                        eT = p2e.tile([128, 2, RPC], bf16, tag="eT", name="eT")
                        nc.scalar.activation(eT[:, :, q0:], sp3[:, :, q0:],
                                             AF.Exp)
                        for hh in range(2):
                            nc.vector.tensor_mul(eT[:, hh, q0:], eT[:, hh, q0:],
                                                 masks_sb[:, kt, q0:])
                        for rc in range(NKV):
                            nc.tensor.matmul(
                                olT_ps[rc][:, :, q0:],
                                c_hat[:, kt, rc * 128:(rc + 1) * 128],
                                eT[:, :, q0:],
                                start=(kt == 0), stop=(kt == NTT - 1),
                                skip_group_check=True)
                        nc.tensor.matmul(
                            den_ps[0:1, :, q0:], ones_sb[:], eT[:, :, q0:],
                            start=(kt == 0), stop=(kt == NTT - 1),
                            skip_group_check=True)
                    # normalize: reciprocal row, broadcast, scale o_v columns
                    den_sb = p2r.tile([1, 2, RPC], f32, tag="den_sb",
                                      name="den_sb")
                    nc.vector.tensor_copy(den_sb[:], den_ps[:])
                    rinv = p2r.tile([1, 2, RPC], f32, tag="rinv", name="rinv")
                    nc.vector.reciprocal(rinv[:], den_sb[:])
                    rb = p2r.tile([128, 2, RPC], f32, tag="rb", name="rb")
                    nc.gpsimd.partition_broadcast(rb[:], rinv[0:1, :, :])
                    olT_sb = p2e.tile([128, NKV, 2, RPC], bf16, tag="olT_sb",
                                      name="olT_sb")
                    for rc in range(NKV):
                        nc.vector.tensor_copy(olT_sb[:, rc, :, :],
                                              olT_ps[rc][:])
                    for hh in range(2):
                        ovp = psp.tile([128, 2 * RPC], f32, tag="sp", name="ovp")
                        for rc in range(NKV):
                            nc.tensor.matmul(
                                ovp[:, 0:RPC], wuv_hh[hh][:, rc, :],
                                olT_sb[:, rc, hh, :],
                                start=(rc == 0), stop=(rc == NKV - 1))
                        nc.vector.tensor_mul(o_vT[:, 2 * hp + hh, :],
                                             ovp[:, 0:RPC], rb[:, hh, :])


        # =================== phase 3: o_proj + residual + post-norm ===========
        if MAXPH >= 3:
            with tc.tile_pool(name="p3", bufs=3) as p3, \
                 tc.tile_pool(name="p3s", bufs=1) as p3s, \
                 tc.tile_pool(name="p3d", bufs=2) as p3d:
                hn = p3s.tile([128, NQT, H], bf16, name="hn")
                with tc.tile_pool(name="p3ps", bufs=2, space="PSUM") as p3ps:
                    op_ps = [p3ps.tile([128, H], f32, tag="opps", name="opps")
                             for _ in range(NQT)]
                    for hc in range(NH):
                        oww = p3.tile([128, H], bf16, tag="oww", name="oww")
                        nc.sync.dma_start(out=oww[:], in_=ow_blk[hc])
                        for qt in range(NQT):
                            for nn in range(4):
                                nc.tensor.matmul(
                                    op_ps[qt][:, nn * 512:(nn + 1) * 512],
                                    o_vT[:, hc, qt * 128:(qt + 1) * 128],
                                    oww[:, nn * 512:(nn + 1) * 512],
                                    start=(hc == 0), stop=(hc == NH - 1))
                    ssq2 = p3s.tile([128, NQT], f32, name="ssq2")
                    for qt in range(NQT):
                        nc.vector.tensor_add(x_rows_sb[:, qt, :],
                                             x_rows_sb[:, qt, :], op_ps[qt][:])
                    for qt in range(NQT):
                        scr = p3d.tile([128, H], bf16, tag="scr3", name="scr3")
                        nc.vector.scalar_tensor_tensor(
                            scr[:], x_rows_sb[:, qt, :], 1.0, x_rows_sb[:, qt, :],
                            ALU.bypass, ALU.mult, accum_out=ssq2[:, qt:qt + 1])
                    nc.scalar.activation(ssq2[:], ssq2[:], AF.Ln, bias=eps_sb[:],
                                         scale=1.0 / H)
                    nc.scalar.activation(ssq2[:], ssq2[:], AF.Exp, scale=-0.5)
                    for qt in range(NQT):
                        nc.vector.tensor_scalar_mul(hn[:, qt, :],
                                                    x_rows_sb[:, qt, :],
                                                    ssq2[:, qt:qt + 1])
                for qt in range(NQT):
                    nc.sync.dma_start_transpose(
                        hnT[:, :, qt * 128:(qt + 1) * 128], hn[:, qt, :])


        # =================== phase 4: MLP ===================
        if MAXPH >= 4:
            with tc.tile_pool(name="p4", bufs=3) as p4, \
                 tc.tile_pool(name="p4ps", bufs=2, space="PSUM") as p4ps:
                for it in range(NIT):
                    gw = p4.tile([128, NFC, 128], bf16, tag="gw", name="gw")
                    nc.sync.dma_start(out=gw[:], in_=gu_blk[0, it])
                    uw = p4.tile([128, NFC, 128], bf16, tag="uw", name="uw")
                    nc.sync.dma_start(out=uw[:], in_=gu_blk[1, it])
                    gp = p4ps.tile([128, RPC], f32, tag="gp", name="gp")
                    up = p4ps.tile([128, RPC], f32, tag="up", name="up")
                    for fc in range(NFC):
                        nc.tensor.matmul(gp[:], gw[:, fc, :], hnT[:, fc, :],
                                         start=(fc == 0), stop=(fc == NFC - 1))
                        nc.tensor.matmul(up[:], uw[:, fc, :], hnT[:, fc, :],
                                         start=(fc == 0), stop=(fc == NFC - 1))
                    gs = p4.tile([128, RPC], bf16, tag="gs", name="gs")
                    nc.scalar.activation(gs[:], gp[:], AF.Silu)
                    nc.vector.tensor_mul(act_all[:, it, :], gs[:], up[:])
            with tc.tile_pool(name="p4b", bufs=3) as p4b, \
                 tc.tile_pool(name="p4s", bufs=2) as p4s, \
                 tc.tile_pool(name="p4bps", bufs=2, space="PSUM") as p4bps:
                o_ps = [p4bps.tile([128, H], f32, tag="ops", name="ops")
                        for _ in range(NQT)]
                for it in range(NIT):
                    dw = p4b.tile([128, H], bf16, tag="dw", name="dw")
                    nc.sync.dma_start(out=dw[:], in_=dw_blk[it])
                    for qt in range(NQT):
                        for nn in range(4):
                            nc.tensor.matmul(
                                o_ps[qt][:, nn * 512:(nn + 1) * 512],
                                act_all[:, it, qt * 128:(qt + 1) * 128],
                                dw[:, nn * 512:(nn + 1) * 512],
                                start=(it == 0), stop=(it == NIT - 1))
                for qt in range(NQT):
                    fin = p4s.tile([128, H], f32, tag="fin", name="fin")
                    nc.vector.tensor_add(fin[:], x_rows_sb[:, qt, :], o_ps[qt][:])
                    nc.sync.dma_start(out=out_rows[qt], in_=fin[:])

        if MAXPH < 4:
            with tc.tile_pool(name="pex", bufs=2) as pex:
                for qt in range(NQT):
                    fin = pex.tile([128, H], f32, tag="finx", name="finx")
                    nc.vector.tensor_copy(fin[:], x_rows_sb[:, qt, :])
                    nc.sync.dma_start(out=out_rows[qt], in_=fin[:])
    nc.compile()
    return nc


def _host_prep(inputs):
    f32 = np.float32
    bf = bfloat16
    x = np.asarray(inputs["hidden_states"], f32)
    pos = np.asarray(inputs["positions"]).astype(f32)

    lnw_in = np.asarray(inputs["input_ln_w"], f32)
    q_a_w = np.asarray(inputs["q_a_w"], f32) * lnw_in[:, None]
    kv_a_w = np.asarray(inputs["kv_a_w"], f32) * lnw_in[:, None]
    q_b_w = (np.asarray(inputs["q_b_w"], f32)
             * np.asarray(inputs["q_a_ln_w"], f32)[:, None]) * SCALE
    kvln = np.asarray(inputs["kv_a_ln_w"], f32)
    w_uk = np.asarray(inputs["w_uk"], f32) * kvln[:, None, None]
    w_uv = np.asarray(inputs["w_uv"], f32) * kvln[:, None, None]
    o_w = np.asarray(inputs["o_w"], f32)
    pln = np.asarray(inputs["post_ln_w"], f32)
    gate_w = np.asarray(inputs["gate_w"], f32) * pln[:, None]
    up_w = np.asarray(inputs["up_w"], f32) * pln[:, None]
    down_w = np.asarray(inputs["down_w"], f32)

    xT = np.ascontiguousarray(x.T)
    inv_freq = 1.0 / (THETA ** (np.arange(0, DR, 2, dtype=f32) / DR))
    ang = pos[:, None] * inv_freq
    cos_t = np.cos(ang).astype(f32)
    sin_t = np.sin(ang).astype(f32)

    gu = np.zeros((2, IPAD, H), f32)
    gu[0, :INTER] = gate_w.T
    gu[1, :INTER] = up_w.T

    rep = {
        "xstat": np.ascontiguousarray(x.reshape(NTT, 128, H).astype(bf)),
        "xT_blk": np.ascontiguousarray(
            xT.astype(bf).reshape(NFC, 128, NTT, 128).transpose(2, 1, 0, 3)),
        "qa_blk": np.ascontiguousarray(q_a_w.astype(bf).reshape(NFC, 128, QLR)),
        # qb_blk[h, rc, p, d] = q_b_w[rc*128+p, h*192+d]
        "qb_blk": np.ascontiguousarray(
            q_b_w.astype(bf).reshape(NRC, 128, NH, QH).transpose(2, 0, 1, 3)),
        "kva_blk": np.ascontiguousarray(
            kv_a_w.astype(bf).reshape(NFC, 128, KVLR + DR)),
        # wuk[h, d, rc, rr] = w_uk[rc*128+rr, h, d]
        "wuk": np.ascontiguousarray(
            w_uk.transpose(1, 2, 0).reshape(NH, 128, NKV, 128).astype(bf)),
        # wuv[h, p, rc, dv] = w_uv[rc*128+p, h, dv]
        "wuv": np.ascontiguousarray(
            w_uv.transpose(1, 0, 2).reshape(NH, NKV, 128, DV)
            .transpose(0, 2, 1, 3).astype(bf)),
        "ow_blk": np.ascontiguousarray(o_w.astype(bf).reshape(NH, 128, H)),
        "gu_blk": np.ascontiguousarray(
            gu.reshape(2, NIT, 128, NFC, 128).transpose(0, 1, 4, 3, 2)
            .astype(bf)),
        "dw_blk": np.ascontiguousarray(
            np.concatenate([down_w, np.zeros((IPAD - INTER, H), f32)], 0)
            .astype(bf).reshape(NIT, 128, H)),
        "cosk": np.ascontiguousarray(
            cos_t.reshape(NTT, 128, DR // 2).transpose(1, 0, 2)),
        "sink": np.ascontiguousarray(
            sin_t.reshape(NTT, 128, DR // 2).transpose(1, 0, 2)),
        "eye": np.eye(128, dtype=bf),
        "ones": np.ones((128, 1), bf),
    }
    # rope pair-swap permutation: out = M @ v; lhsT = M.T
    M = np.zeros((DR, DR), f32)
    for i in range(DR // 2):
        M[2 * i, 2 * i + 1] = -1.0
        M[2 * i + 1, 2 * i] = 1.0
    rep["permT"] = np.ascontiguousarray(M.T).astype(bf)

    per_core = []
    for c in range(NCORES):
        rows = np.arange(c, T, NCORES)
        m = dict(rep)
        m["x_rows"] = np.ascontiguousarray(x[rows].reshape(NQT, 128, H))
        m["xTc"] = np.ascontiguousarray(
            xT[:, rows].astype(bf).reshape(NFC, 128, RPC))
        # [64, RPC] rope tables, row d -> freq d//2
        m["cosqT"] = np.ascontiguousarray(
            np.repeat(cos_t[rows].T, 2, axis=0).astype(f32))
        m["sinqT"] = np.ascontiguousarray(
            np.repeat(sin_t[rows].T, 2, axis=0).astype(f32))
        mask = np.zeros((NTT, 128, RPC), f32)
        kpos = np.arange(128)
        for kt in range(NTT):
            gk = kt * 128 + kpos
            mask[kt] = (gk[:, None] <= rows[None, :]).astype(f32)
        m["masks"] = mask.astype(bf)
        per_core.append(m)
    return per_core


def kernel(**inputs):
    from concourse import bass_utils

    if "nc" not in _CACHE:
        _CACHE["nc"] = _build_module()
    nc = _CACHE["nc"]

    import os
    in_maps = _host_prep(inputs)
    trace = bool(os.environ.get("BASS_KERNEL_TRACE"))
    res = bass_utils.run_bass_kernel_spmd(nc, in_maps,
                                          core_ids=list(range(NCORES)),
                                          trace=trace)
    if trace and res.exec_time_ns is not None:
        print(f"HW exec time: {res.exec_time_ns} ns")
        _CACHE["last_result"] = res
    out = np.zeros((T, H), np.float32)
    for c in range(NCORES):
        rows = np.arange(c, T, NCORES)
        out[rows] = res.results[c]["out_rows"].reshape(RPC, H)
    return out



# revision 30
# speedup vs baseline: 1.0652x; 1.0652x over previous
"""DeepseekV3 decoder layer (MLA + SwiGLU MLP), T=2048 prefill, fp32 I/O.

Sharding: sequence-parallel striped — core c owns token rows c::8 (256 rows),
so all 8 cores run one identical SPMD program with balanced causal work; only
input data differs per core. The KV latent path (all 2048 tokens) is
replicated on every core; outputs are disjoint row sets concatenated on host.

Per core: row-major activations (per-token norm scales are per-partition),
bf16 matmul operands with fp32 PSUM accumulation, LN weights folded into
adjacent GEMMs on host, RoPE via host cos/sin tables, softmax without
max-subtraction (scores are O(30) max), denominator via ones-matmul,
causal masking by static tile skipping + mask multiply. The q_b/absorb
path runs lazily per head inside the attention loop to bound SBUF.
"""

import numpy as np
import ml_dtypes

bfloat16 = ml_dtypes.bfloat16

T = 2048
H = 2048
NH = 16
QLR = 1536
KVLR = 512
DN = 128
DR = 64
DV = 128
INTER = 10944
NCORES = 8
RPC = T // NCORES
NQT = RPC // 128
NTT = T // 128
NFC = H // 128
NRC = QLR // 128
NKV = KVLR // 128
NIT = 86
IPAD = NIT * 128
EPS = 1e-6
SCALE = (DN + DR) ** -0.5
THETA = 10000.0
QH = DN + DR               # 192 per-head q dim

_CACHE = {}


def _build_module():
    import os
    MAXPH = int(os.environ.get("KERNEL_MAXPH", "9"))
    import concourse.bass as bass
    import concourse.tile as tile
    from concourse import bacc, mybir

    f32 = mybir.dt.float32
    bf16 = mybir.dt.bfloat16
    AF = mybir.ActivationFunctionType
    ALU = mybir.AluOpType

    nc = bacc.Bacc("TRN2", target_bir_lowering=False, debug=False,
                   enable_asserts=False, num_devices=NCORES)

    def inp(name, shape, dt):
        return nc.dram_tensor(name, list(shape), dt, kind="ExternalInput").ap()

    # per-core inputs
    x_rows = inp("x_rows", [NQT, 128, H], f32)
    xTc = inp("xTc", [NFC, 128, RPC], bf16)
    cosqT = inp("cosqT", [64, RPC], f32)
    sinqT = inp("sinqT", [64, RPC], f32)
    masks = inp("masks", [NTT, 128, RPC], bf16)
    permT = inp("permT", [64, 64], bf16)
    # replicated inputs
    xstat = inp("xstat", [NTT, 128, H], bf16)
    xT_blk = inp("xT_blk", [NTT, 128, NFC, 128], bf16)
    qa_blk = inp("qa_blk", [NFC, 128, QLR], bf16)
    qb_blk = inp("qb_blk", [NH, NRC, 128, QH], bf16)
    kva_blk = inp("kva_blk", [NFC, 128, KVLR + DR], bf16)
    wuk = inp("wuk", [NH, 128, NKV, 128], bf16)
    wuv = inp("wuv", [NH, 128, NKV, DV], bf16)
    ow_blk = inp("ow_blk", [NH, 128, H], bf16)
    gu_blk = inp("gu_blk", [2, NIT, 128, NFC, 128], bf16)
    dw_blk = inp("dw_blk", [NIT, 128, H], bf16)
    cosk = inp("cosk", [128, NTT, DR // 2], f32)
    sink = inp("sink", [128, NTT, DR // 2], f32)
    eye = inp("eye", [128, 128], bf16)
    ones = inp("ones", [128, 1], bf16)

    out_rows = nc.dram_tensor("out_rows", [NQT, 128, H], f32,
                              kind="ExternalOutput").ap()

    from contextlib import ExitStack
    with tile.TileContext(nc) as tc, ExitStack() as ctx:
        persist = ctx.enter_context(tc.tile_pool(name="persist", bufs=1))

        def pt(shape, dt, tag):
            return persist.tile(list(shape), dt, tag=tag, name=tag)

        eps_sb = pt([128, 1], f32, "eps")
        nc.vector.memset(eps_sb[:], EPS)
        eye_sb = pt([128, 128], bf16, "eye")
        nc.sync.dma_start(out=eye_sb[:], in_=eye[:])
        ones_sb = pt([128, 1], bf16, "ones")
        nc.sync.dma_start(out=ones_sb[:], in_=ones[:])
        x_rows_sb = pt([128, NQT, H], f32, "x_rows")
        for qt in range(NQT):
            nc.sync.dma_start(out=x_rows_sb[:, qt, :], in_=x_rows[qt])

        rstd_all = pt([128, NTT], f32, "rstd_all")
        s_ck = pt([128, NTT], f32, "s_ck")
        c_hat = pt([128, NTT, KVLR], bf16, "c_hat")
        kT_lat = pt([128, NKV, T], bf16, "kT_lat")
        kT_rope = pt([64, T], bf16, "kT_rope")
        qcT = pt([128, NRC, RPC], bf16, "qcT")
        o_vT = pt([128, NH, RPC], bf16, "o_vT")
        hnT = pt([128, NFC, RPC], bf16, "hnT")
        act_all = pt([128, NIT, RPC], bf16, "act_all")

        # ========== phases 0+1 interleaved: kv path + q_a ==========
        # Phase 1's GEMM is emitted between phase 0's kv GEMM and the kT
        # transposes so the PE stays busy while the kv norm/rope chain runs
        # on Vector/Scalar.
        with tc.tile_pool(name="p0s", bufs=1) as p0s, \
             tc.tile_pool(name="p0d", bufs=1) as p0d:
            cosk_sb = p0s.tile([128, NTT, DR // 2], f32, name="cosk_sb")
            nc.sync.dma_start(out=cosk_sb[:], in_=cosk[:])
            sink_sb = p0s.tile([128, NTT, DR // 2], f32, name="sink_sb")
            nc.sync.dma_start(out=sink_sb[:], in_=sink[:])
            ssq_all = p0s.tile([128, NTT], f32, name="ssq_all")
            ssq_kv = p0s.tile([128, NTT], f32, name="ssq_kv")
            c_raw = p0s.tile([128, NTT, KVLR + DR], bf16, name="c_raw")
            with tc.tile_pool(name="p0a", bufs=2) as p0a, \
                 tc.tile_pool(name="p0w", bufs=NFC) as p0w, \
                 tc.tile_pool(name="p0ps", bufs=2, space="PSUM") as p0ps:
                for tt in range(NTT):
                    xs = p0a.tile([128, H], bf16, tag="xs", name="xs")
                    nc.gpsimd.dma_start(out=xs[:], in_=xstat[tt])
                    scrap = p0d.tile([128, H], bf16, tag="scrap", name="scrap")
                    nc.vector.scalar_tensor_tensor(
                        scrap[:], xs[:], 1.0, xs[:], ALU.bypass, ALU.mult,
                        accum_out=ssq_all[:, tt:tt + 1])
                nc.scalar.activation(rstd_all[:], ssq_all[:], AF.Ln,
                                     bias=eps_sb[:], scale=1.0 / H)
                nc.scalar.activation(rstd_all[:], rstd_all[:], AF.Exp,
                                     scale=-0.5)

                kvw = []
                for fc in range(NFC):
                    w = p0w.tile([128, KVLR + DR], bf16, tag="kvw", name="kvw")
                    nc.sync.dma_start(out=w[:], in_=kva_blk[fc])
                    kvw.append(w)
                for tt in range(NTT):
                    xt = p0a.tile([128, NFC, 128], bf16, tag="xt", name="xt")
                    nc.gpsimd.dma_start(out=xt[:], in_=xT_blk[tt])
                    ps = p0ps.tile([128, KVLR + DR], f32, tag="kvps",
                                   name="kvps")
                    for fc in range(NFC):
                        nc.tensor.matmul(ps[:, 0:512], xt[:, fc, :],
                                         kvw[fc][:, 0:512],
                                         start=(fc == 0), stop=(fc == NFC - 1))
                        nc.tensor.matmul(ps[:, 512:576], xt[:, fc, :],
                                         kvw[fc][:, 512:576],
                                         start=(fc == 0), stop=(fc == NFC - 1))
                    scr2 = p0d.tile([128, KVLR], bf16, tag="scr2", name="scr2")
                    nc.scalar.activation(scr2[:], ps[:, 0:512], AF.Square,
                                         accum_out=ssq_kv[:, tt:tt + 1])
                    nc.vector.tensor_copy(c_raw[:, tt, :], ps[:])
            t1 = p0s.tile([128, NTT], f32, name="t1")
            nc.vector.tensor_mul(t1[:], rstd_all[:], rstd_all[:])
            nc.vector.tensor_mul(t1[:], t1[:], ssq_kv[:])
            nc.scalar.activation(t1[:], t1[:], AF.Ln, bias=eps_sb[:],
                                 scale=1.0 / KVLR)
            nc.scalar.activation(t1[:], t1[:], AF.Exp, scale=-0.5)
            nc.vector.tensor_mul(s_ck[:], rstd_all[:], t1[:])
            for tt in range(NTT):
                nc.vector.tensor_scalar_mul(c_hat[:, tt, :], c_raw[:, tt, 0:512],
                                            s_ck[:, tt:tt + 1])
            kr = p0s.tile([128, NTT, DR], bf16, name="kr")
            krf = p0s.tile([128, NTT, DR], bf16, name="krf")
            for tt in range(NTT):
                nc.vector.tensor_scalar_mul(kr[:, tt, :], c_raw[:, tt, 512:576],
                                            rstd_all[:, tt:tt + 1])
            x1 = kr[:, :, 0:DR:2]
            x2 = kr[:, :, 1:DR:2]
            ta = p0s.tile([128, NTT, DR // 2], f32, name="ta")
            tb = p0s.tile([128, NTT, DR // 2], f32, name="tb")
            nc.vector.tensor_mul(ta[:], x1, cosk_sb[:])
            nc.vector.tensor_mul(tb[:], x2, sink_sb[:])
            nc.vector.tensor_sub(krf[:, :, 0:DR:2], ta[:], tb[:])
            nc.vector.tensor_mul(ta[:], x2, cosk_sb[:])
            nc.vector.tensor_mul(tb[:], x1, sink_sb[:])
            nc.vector.tensor_add(krf[:, :, 1:DR:2], ta[:], tb[:])

            # ---- phase 1 (q_a -> qcT) emitted here: its GEMM keeps the PE
            # busy while the kv norm/rope chain above runs on Vector/Scalar
            if MAXPH >= 1:
                with tc.tile_pool(name="p1", bufs=3) as p1, \
                     tc.tile_pool(name="p1s", bufs=1) as p1s, \
                     tc.tile_pool(name="p1d", bufs=1) as p1d, \
                     tc.tile_pool(name="p1ps", bufs=2, space="PSUM") as p1ps, \
                     tc.tile_pool(name="p1tp", bufs=2, space="PSUM") as p1tp:
                    rstd_rows = p1s.tile([128, NQT], f32, name="rstd_rows")
                    ssq_r = p1s.tile([128, NQT], f32, name="ssq_r")
                    for qt in range(NQT):
                        scrap = p1d.tile([128, H], bf16, tag="scrapq",
                                         name="scrapq")
                        nc.vector.scalar_tensor_tensor(
                            scrap[:], x_rows_sb[:, qt, :], 1.0,
                            x_rows_sb[:, qt, :],
                            ALU.bypass, ALU.mult, accum_out=ssq_r[:, qt:qt + 1])
                    nc.scalar.activation(rstd_rows[:], ssq_r[:], AF.Ln,
                                         bias=eps_sb[:], scale=1.0 / H)
                    nc.scalar.activation(rstd_rows[:], rstd_rows[:], AF.Exp,
                                         scale=-0.5)

                    xTc_sb = p1s.tile([128, NFC, RPC], bf16, name="xTc_sb")
                    for fc in range(NFC):
                        nc.sync.dma_start(out=xTc_sb[:, fc, :], in_=xTc[fc])
                    qa_ps = [p1ps.tile([128, QLR], f32, tag="mm", name="mm")
                             for _ in range(NQT)]
                    for fc in range(NFC):
                        qaw = p1.tile([128, QLR], bf16, tag="qaw", name="qaw")
                        nc.sync.dma_start(out=qaw[:], in_=qa_blk[fc])
                        for qt in range(NQT):
                            for nn in range(QLR // 512):
                                nc.tensor.matmul(
                                    qa_ps[qt][:, nn * 512:(nn + 1) * 512],
                                    xTc_sb[:, fc, qt * 128:(qt + 1) * 128],
                                    qaw[:, nn * 512:(nn + 1) * 512],
                                    start=(fc == 0), stop=(fc == NFC - 1))
                    qc = p1s.tile([128, NQT, QLR], bf16, name="qc")
                    ssq_q = p1s.tile([128, NQT], f32, name="ssq_q")
                    for qt in range(NQT):
                        scr = p1d.tile([128, QLR], bf16, tag="scrq2",
                                       name="scrq2")
                        nc.scalar.activation(scr[:], qa_ps[qt][:], AF.Square,
                                             accum_out=ssq_q[:, qt:qt + 1])
                    sq = p1s.tile([128, NQT], f32, name="sq")
                    nc.vector.tensor_mul(sq[:], rstd_rows[:], rstd_rows[:])
                    nc.vector.tensor_mul(sq[:], sq[:], ssq_q[:])
                    nc.scalar.activation(sq[:], sq[:], AF.Ln, bias=eps_sb[:],
                                         scale=1.0 / QLR)
                    nc.scalar.activation(sq[:], sq[:], AF.Exp, scale=-0.5)
                    nc.vector.tensor_mul(sq[:], rstd_rows[:], sq[:])
                    for qt in range(NQT):
                        nc.vector.tensor_scalar_mul(qc[:, qt, :], qa_ps[qt][:],
                                                    sq[:, qt:qt + 1])
                    for qt in range(NQT):
                        for rc in range(NRC):
                            tp = p1tp.tile([128, 128], bf16, tag="tp",
                                           name="tp")
                            nc.tensor.transpose(
                                tp[:], qc[:, qt, rc * 128:(rc + 1) * 128],
                                eye_sb[:])
                            nc.any.tensor_copy(
                                qcT[:, rc, qt * 128:(qt + 1) * 128], tp[:])

            # ---- kT transposes after phase 1: PE-ordered behind it, and
            # phase 2's first q_b matmuls only need qcT anyway
            with tc.tile_pool(name="p0tp", bufs=2, space="PSUM") as p0tp:
                for tt in range(NTT):
                    for rc in range(NKV):
                        tp = p0tp.tile([128, 128], bf16, tag="tp", name="tp")
                        nc.tensor.transpose(
                            tp[:], c_hat[:, tt, rc * 128:(rc + 1) * 128],
                            eye_sb[:])
                        nc.any.tensor_copy(
                            kT_lat[:, rc, tt * 128:(tt + 1) * 128], tp[:])
                    tp = p0tp.tile([128, 128], bf16, tag="tp", name="tp")
                    nc.tensor.transpose(tp[0:64, :], krf[:, tt, :], eye_sb[:])
                    nc.any.tensor_copy(kT_rope[:, tt * 128:(tt + 1) * 128],
                                       tp[0:64, :])

        # ============ phase 2: head-pair q_b + attention ============
        # Heads processed in pairs: the kT_lat/kT_rope/c_hat stationaries are
        # shared across heads, so a 3D moving AP [128, 2, N] doubles the free
        # dim per LDWEIGHTS and halves the LDW count (the prior bottleneck).
        if MAXPH >= 2:
            with tc.tile_pool(name="p2", bufs=4) as p2, \
                 tc.tile_pool(name="p2s", bufs=1) as p2s, \
                 tc.tile_pool(name="p2d", bufs=2) as p2d, \
                 tc.tile_pool(name="p2e", bufs=4) as p2e, \
                 tc.tile_pool(name="p2r", bufs=2) as p2r, \
                 tc.tile_pool(name="pQ", bufs=1, space="PSUM") as pQ, \
                 tc.tile_pool(name="psp", bufs=2, space="PSUM") as psp, \
                 tc.tile_pool(name="pO", bufs=4, space="PSUM") as pO, \
                 tc.tile_pool(name="pD", bufs=1, space="PSUM") as pD:
                masks_sb = p2s.tile([128, NTT, RPC], bf16, name="masks_sb")
                for kt in range(NTT):
                    nc.sync.dma_start(out=masks_sb[:, kt, :], in_=masks[kt])
                cosqT_sb = p2s.tile([64, RPC], f32, name="cosqT_sb")
                nc.sync.dma_start(out=cosqT_sb[:], in_=cosqT[:])
                sinqT_sb = p2s.tile([64, RPC], f32, name="sinqT_sb")
                nc.sync.dma_start(out=sinqT_sb[:], in_=sinqT[:])
                permT_sb = p2s.tile([64, 64], bf16, name="permT_sb")
                nc.sync.dma_start(out=permT_sb[:], in_=permT[:])

                for hp in range(NH // 2):
                    qT2 = p2d.tile([128, NKV, 2, RPC], bf16, tag="qT2",
                                   name="qT2")
                    qrT2 = p2d.tile([64, 2, RPC], bf16, tag="qrT2", name="qrT2")
                    wuv_hh = []
                    for hh in range(2):
                        h = 2 * hp + hh
                        qbw = p2.tile([128, NRC, QH], bf16, tag="qbw",
                                      name="qbw")
                        for rc in range(NRC):
                            nc.sync.dma_start(out=qbw[:, rc, :],
                                              in_=qb_blk[h, rc])
                        wuk_h = p2.tile([128, NKV, 128], bf16, tag="wuk_h",
                                        name="wuk_h")
                        nc.sync.dma_start(out=wuk_h[:], in_=wuk[h])
                        wuv_h = p2.tile([128, NKV, DV], bf16, tag="wuv_h",
                                        name="wuv_h")
                        nc.sync.dma_start(out=wuv_h[:], in_=wuv[h])
                        wuv_hh.append(wuv_h)

                        # q_b transposed: nope [dn, tok] + rope [dr, tok]
                        # share one PSUM bank; only the first MM carries start
                        qnr_ps = pQ.tile([128, 512], f32, tag="qnr", name="qnr")
                        for rc in range(NRC):
                            nc.tensor.matmul(
                                qnr_ps[:, 0:RPC], qbw[:, rc, 0:DN],
                                qcT[:, rc, :],
                                start=(rc == 0), stop=(rc == NRC - 1),
                                skip_group_check=True)
                            nc.tensor.matmul(
                                qnr_ps[0:DR, RPC:2 * RPC], qbw[:, rc, DN:QH],
                                qcT[:, rc, :],
                                start=False, stop=(rc == NRC - 1),
                                skip_group_check=True)
                        qnT = p2d.tile([128, RPC], bf16, tag="qnT", name="qnT")
                        nc.vector.tensor_copy(qnT[:], qnr_ps[:, 0:RPC])
                        # rope via signed pair-swap perm matmul: C*x + P(S*x)
                        tS = p2d.tile([64, RPC], bf16, tag="tS", name="tS")
                        nc.vector.tensor_mul(tS[:], qnr_ps[0:DR, RPC:2 * RPC],
                                             sinqT_sb[:])
                        tC = p2d.tile([64, RPC], f32, tag="tC", name="tC")
                        nc.vector.tensor_mul(tC[:], qnr_ps[0:DR, RPC:2 * RPC],
                                             cosqT_sb[:])
                        pr_ps = psp.tile([128, 2 * RPC], f32, tag="sp",
                                         name="prps")
                        nc.tensor.matmul(pr_ps[0:DR, 0:RPC], permT_sb[:], tS[:],
                                         start=True, stop=True)
                        nc.vector.tensor_add(qrT2[:, hh, :], tC[:],
                                             pr_ps[0:DR, 0:RPC])
                        # absorb w_uk into q
                        for rc in range(NKV):
                            lp = psp.tile([128, 2 * RPC], f32, tag="sp",
                                          name="lp")
                            nc.tensor.matmul(lp[:, 0:RPC], wuk_h[:, rc, :],
                                             qnT[:], start=True, stop=True)
                            nc.vector.tensor_copy(qT2[:, rc, hh, :],
                                                  lp[:, 0:RPC])

                    olT_ps = [pO.tile([128, 2, RPC], f32, tag="olT", name="olT")
                              for _ in range(NKV)]
                    den_ps = pD.tile([1, 2, RPC], f32, tag="den", name="den")
                    for kt in range(NTT):
                        q0 = 16 * kt
                        sp3 = psp.tile([128, 2, RPC], f32, tag="sp", name="sp")
                        for dc in range(NKV):
                            nc.tensor.matmul(
                                sp3[:, :, q0:],
                                kT_lat[:, dc, kt * 128:(kt + 1) * 128],
                                qT2[:, dc, :, q0:],
                                start=(dc == 0), stop=False,
                                skip_group_check=True)
                        nc.tensor.matmul(
                            sp3[:, :, q0:],
                            kT_rope[:, kt * 128:(kt + 1) * 128],
                            qrT2[:, :, q0:],
                            start=False, stop=True, skip_group_check=True)
                        eT = p2e.tile([128, 2, RPC], bf16, tag="eT", name="eT")
                        nc.scalar.activation(eT[:, :, q0:], sp3[:, :, q0:],
                                             AF.Exp)
                        for hh in range(2):
                            nc.vector.tensor_mul(eT[:, hh, q0:], eT[:, hh, q0:],
                                                 masks_sb[:, kt, q0:])
                        for rc in range(NKV):
                            nc.tensor.matmul(
                                olT_ps[rc][:, :, q0:],
                                c_hat[:, kt, rc * 128:(rc + 1) * 128],
                                eT[:, :, q0:],
                                start=(kt == 0), stop=(kt == NTT - 1),
                                skip_group_check=True)
                        nc.tensor.matmul(
                            den_ps[0:1, :, q0:], ones_sb[:], eT[:, :, q0:],
                            start=(kt == 0), stop=(kt == NTT - 1),
                            skip_group_check=True)
                    # normalize: reciprocal row, broadcast, scale o_v columns
                    den_sb = p2r.tile([1, 2, RPC], f32, tag="den_sb",
                                      name="den_sb")
                    nc.vector.tensor_copy(den_sb[:], den_ps[:])
                    rinv = p2r.tile([1, 2, RPC], f32, tag="rinv", name="rinv")
                    nc.vector.reciprocal(rinv[:], den_sb[:])
                    rb = p2r.tile([128, 2, RPC], f32, tag="rb", name="rb")
                    nc.gpsimd.partition_broadcast(rb[:], rinv[0:1, :, :])
                    olT_sb = p2e.tile([128, NKV, 2, RPC], bf16, tag="olT_sb",
                                      name="olT_sb")
                    for rc in range(NKV):
                        nc.vector.tensor_copy(olT_sb[:, rc, :, :],
                                              olT_ps[rc][:])
                    for hh in range(2):
                        ovp = psp.tile([128, 2 * RPC], f32, tag="sp", name="ovp")
                        for rc in range(NKV):
                            nc.tensor.matmul(
                                ovp[:, 0:RPC], wuv_hh[hh][:, rc, :],
                                olT_sb[:, rc, hh, :],
                                start=(rc == 0), stop=(rc == NKV - 1))
                        nc.vector.tensor_mul(o_vT[:, 2 * hp + hh, :],
                                             ovp[:, 0:RPC], rb[:, hh, :])


        # =================== phase 3: o_proj + residual + post-norm ===========
        if MAXPH >= 3:
            with tc.tile_pool(name="p3", bufs=3) as p3, \
                 tc.tile_pool(name="p3s", bufs=1) as p3s, \
                 tc.tile_pool(name="p3d", bufs=2) as p3d:
                hn = p3s.tile([128, NQT, H], bf16, name="hn")
                with tc.tile_pool(name="p3ps", bufs=2, space="PSUM") as p3ps:
                    op_ps = [p3ps.tile([128, H], f32, tag="opps", name="opps")
                             for _ in range(NQT)]
                    for hc in range(NH):
                        oww = p3.tile([128, H], bf16, tag="oww", name="oww")
                        nc.sync.dma_start(out=oww[:], in_=ow_blk[hc])
                        for qt in range(NQT):
                            for nn in range(4):
                                nc.tensor.matmul(
                                    op_ps[qt][:, nn * 512:(nn + 1) * 512],
                                    o_vT[:, hc, qt * 128:(qt + 1) * 128],
                                    oww[:, nn * 512:(nn + 1) * 512],
                                    start=(hc == 0), stop=(hc == NH - 1))
                    ssq2 = p3s.tile([128, NQT], f32, name="ssq2")
                    for qt in range(NQT):
                        nc.vector.tensor_add(x_rows_sb[:, qt, :],
                                             x_rows_sb[:, qt, :], op_ps[qt][:])
                    for qt in range(NQT):
                        scr = p3d.tile([128, H], bf16, tag="scr3", name="scr3")
                        nc.vector.scalar_tensor_tensor(
                            scr[:], x_rows_sb[:, qt, :], 1.0, x_rows_sb[:, qt, :],
                            ALU.bypass, ALU.mult, accum_out=ssq2[:, qt:qt + 1])
                    nc.scalar.activation(ssq2[:], ssq2[:], AF.Ln, bias=eps_sb[:],
                                         scale=1.0 / H)
                    nc.scalar.activation(ssq2[:], ssq2[:], AF.Exp, scale=-0.5)
                    for qt in range(NQT):
                        nc.vector.tensor_scalar_mul(hn[:, qt, :],
                                                    x_rows_sb[:, qt, :],
                                                    ssq2[:, qt:qt + 1])
                for qt in range(NQT):
                    nc.sync.dma_start_transpose(
                        hnT[:, :, qt * 128:(qt + 1) * 128], hn[:, qt, :])


        # =================== phase 4: MLP ===================
        # dw tiles stream on the scalar-engine HWDGE ring (gw/uw own the sync
        # ring), with the first NPRE emitted inside the 4a loop so the down
        # weights prefetch while gate/up still computes.
        if MAXPH >= 4:
            NPRE = 6
            with tc.tile_pool(name="p4b", bufs=NPRE) as p4b, \
                 tc.tile_pool(name="p4s", bufs=2) as p4s:
                dw_tiles = {}

                def dw_fetch(it):
                    dw = p4b.tile([128, H], bf16, tag="dw", name="dw")
                    nc.scalar.dma_start(out=dw[:], in_=dw_blk[it])
                    dw_tiles[it] = dw

                with tc.tile_pool(name="p4", bufs=3) as p4, \
                     tc.tile_pool(name="p4ps", bufs=2, space="PSUM") as p4ps:
                    for it in range(NIT):
                        gw = p4.tile([128, NFC, 128], bf16, tag="gw", name="gw")
                        nc.sync.dma_start(out=gw[:], in_=gu_blk[0, it])
                        uw = p4.tile([128, NFC, 128], bf16, tag="uw", name="uw")
                        nc.sync.dma_start(out=uw[:], in_=gu_blk[1, it])
                        gp = p4ps.tile([128, RPC], f32, tag="gp", name="gp")
                        up = p4ps.tile([128, RPC], f32, tag="up", name="up")
                        for fc in range(NFC):
                            nc.tensor.matmul(gp[:], gw[:, fc, :], hnT[:, fc, :],
                                             start=(fc == 0),
                                             stop=(fc == NFC - 1))
                            nc.tensor.matmul(up[:], uw[:, fc, :], hnT[:, fc, :],
                                             start=(fc == 0),
                                             stop=(fc == NFC - 1))
                        gs = p4.tile([128, RPC], bf16, tag="gs", name="gs")
                        nc.scalar.activation(gs[:], gp[:], AF.Silu)
                        nc.vector.tensor_mul(act_all[:, it, :], gs[:], up[:])
                        if NIT - NPRE <= it < NIT:
                            dw_fetch(it - (NIT - NPRE))
                with tc.tile_pool(name="p4bps", bufs=2, space="PSUM") as p4bps:
                    o_ps = [p4bps.tile([128, H], f32, tag="ops", name="ops")
                            for _ in range(NQT)]
                    for it in range(NIT):
                        if it not in dw_tiles:
                            dw_fetch(it)
                        dw = dw_tiles.pop(it)
                        for qt in range(NQT):
                            for nn in range(4):
                                nc.tensor.matmul(
                                    o_ps[qt][:, nn * 512:(nn + 1) * 512],
                                    act_all[:, it, qt * 128:(qt + 1) * 128],
                                    dw[:, nn * 512:(nn + 1) * 512],
                                    start=(it == 0), stop=(it == NIT - 1))
                    for qt in range(NQT):
                        fin = p4s.tile([128, H], f32, tag="fin", name="fin")
                        nc.vector.tensor_add(fin[:], x_rows_sb[:, qt, :],
                                             o_ps[qt][:])
                        nc.sync.dma_start(out=out_rows[qt], in_=fin[:])

        if MAXPH < 4:
            with tc.tile_pool(name="pex", bufs=2) as pex:
                for qt in range(NQT):
                    fin = pex.tile([128, H], f32, tag="finx", name="finx")
                    nc.vector.tensor_copy(fin[:], x_rows_sb[:, qt, :])
                    nc.sync.dma_start(out=out_rows[qt], in_=fin[:])
    nc.compile()
    return nc


def _host_prep(inputs):
    f32 = np.float32
    bf = bfloat16
    x = np.asarray(inputs["hidden_states"], f32)
    pos = np.asarray(inputs["positions"]).astype(f32)

    lnw_in = np.asarray(inputs["input_ln_w"], f32)
    q_a_w = np.asarray(inputs["q_a_w"], f32) * lnw_in[:, None]
    kv_a_w = np.asarray(inputs["kv_a_w"], f32) * lnw_in[:, None]
    q_b_w = (np.asarray(inputs["q_b_w"], f32)
             * np.asarray(inputs["q_a_ln_w"], f32)[:, None]) * SCALE
    kvln = np.asarray(inputs["kv_a_ln_w"], f32)
    w_uk = np.asarray(inputs["w_uk"], f32) * kvln[:, None, None]
    w_uv = np.asarray(inputs["w_uv"], f32) * kvln[:, None, None]
    o_w = np.asarray(inputs["o_w"], f32)
    pln = np.asarray(inputs["post_ln_w"], f32)
    gate_w = np.asarray(inputs["gate_w"], f32) * pln[:, None]
    up_w = np.asarray(inputs["up_w"], f32) * pln[:, None]
    down_w = np.asarray(inputs["down_w"], f32)

    xT = np.ascontiguousarray(x.T)
    inv_freq = 1.0 / (THETA ** (np.arange(0, DR, 2, dtype=f32) / DR))
    ang = pos[:, None] * inv_freq
    cos_t = np.cos(ang).astype(f32)
    sin_t = np.sin(ang).astype(f32)

    gu = np.zeros((2, IPAD, H), f32)
    gu[0, :INTER] = gate_w.T
    gu[1, :INTER] = up_w.T

    rep = {
        "xstat": np.ascontiguousarray(x.reshape(NTT, 128, H).astype(bf)),
        "xT_blk": np.ascontiguousarray(
            xT.astype(bf).reshape(NFC, 128, NTT, 128).transpose(2, 1, 0, 3)),
        "qa_blk": np.ascontiguousarray(q_a_w.astype(bf).reshape(NFC, 128, QLR)),
        # qb_blk[h, rc, p, d] = q_b_w[rc*128+p, h*192+d]
        "qb_blk": np.ascontiguousarray(
            q_b_w.astype(bf).reshape(NRC, 128, NH, QH).transpose(2, 0, 1, 3)),
        "kva_blk": np.ascontiguousarray(
            kv_a_w.astype(bf).reshape(NFC, 128, KVLR + DR)),
        # wuk[h, d, rc, rr] = w_uk[rc*128+rr, h, d]
        "wuk": np.ascontiguousarray(
            w_uk.transpose(1, 2, 0).reshape(NH, 128, NKV, 128).astype(bf)),
        # wuv[h, p, rc, dv] = w_uv[rc*128+p, h, dv]
        "wuv": np.ascontiguousarray(
            w_uv.transpose(1, 0, 2).reshape(NH, NKV, 128, DV)
            .transpose(0, 2, 1, 3).astype(bf)),
        "ow_blk": np.ascontiguousarray(o_w.astype(bf).reshape(NH, 128, H)),
        "gu_blk": np.ascontiguousarray(
            gu.reshape(2, NIT, 128, NFC, 128).transpose(0, 1, 4, 3, 2)
            .astype(bf)),
        "dw_blk": np.ascontiguousarray(
            np.concatenate([down_w, np.zeros((IPAD - INTER, H), f32)], 0)
            .astype(bf).reshape(NIT, 128, H)),
        "cosk": np.ascontiguousarray(
            cos_t.reshape(NTT, 128, DR // 2).transpose(1, 0, 2)),
        "sink": np.ascontiguousarray(
            sin_t.reshape(NTT, 128, DR // 2).transpose(1, 0, 2)),
        "eye": np.eye(128, dtype=bf),
        "ones": np.ones((128, 1), bf),
    }
    # rope pair-swap permutation: out = M @ v; lhsT = M.T
    M = np.zeros((DR, DR), f32)
    for i in range(DR // 2):
        M[2 * i, 2 * i + 1] = -1.0
        M[2 * i + 1, 2 * i] = 1.0
    rep["permT"] = np.ascontiguousarray(M.T).astype(bf)

    per_core = []
    for c in range(NCORES):
        rows = np.arange(c, T, NCORES)
        m = dict(rep)
        m["x_rows"] = np.ascontiguousarray(x[rows].reshape(NQT, 128, H))
        m["xTc"] = np.ascontiguousarray(
            xT[:, rows].astype(bf).reshape(NFC, 128, RPC))
        # [64, RPC] rope tables, row d -> freq d//2
        m["cosqT"] = np.ascontiguousarray(
            np.repeat(cos_t[rows].T, 2, axis=0).astype(f32))
        m["sinqT"] = np.ascontiguousarray(
            np.repeat(sin_t[rows].T, 2, axis=0).astype(f32))
        mask = np.zeros((NTT, 128, RPC), f32)
        kpos = np.arange(128)
        for kt in range(NTT):
            gk = kt * 128 + kpos
            mask[kt] = (gk[:, None] <= rows[None, :]).astype(f32)
        m["masks"] = mask.astype(bf)
        per_core.append(m)
    return per_core


def kernel(**inputs):
    from concourse import bass_utils

    if "nc" not in _CACHE:
        _CACHE["nc"] = _build_module()
    nc = _CACHE["nc"]

    import os
    in_maps = _host_prep(inputs)
    trace = bool(os.environ.get("BASS_KERNEL_TRACE"))
    res = bass_utils.run_bass_kernel_spmd(nc, in_maps,
                                          core_ids=list(range(NCORES)),
                                          trace=trace)
    if trace and res.exec_time_ns is not None:
        print(f"HW exec time: {res.exec_time_ns} ns")
        _CACHE["last_result"] = res
    out = np.zeros((T, H), np.float32)
    for c in range(NCORES):
        rows = np.arange(c, T, NCORES)
        out[rows] = res.results[c]["out_rows"].reshape(RPC, H)
    return out



# revision 32
# speedup vs baseline: 1.0862x; 1.0197x over previous
"""DeepseekV3 decoder layer (MLA + SwiGLU MLP), T=2048 prefill, fp32 I/O.

Sharding: sequence-parallel striped — core c owns token rows c::8 (256 rows),
so all 8 cores run one identical SPMD program with balanced causal work; only
input data differs per core. The KV latent path (all 2048 tokens) is
replicated on every core; outputs are disjoint row sets concatenated on host.

Per core: row-major activations (per-token norm scales are per-partition),
bf16 matmul operands with fp32 PSUM accumulation, LN weights folded into
adjacent GEMMs on host, RoPE via host cos/sin tables, softmax without
max-subtraction (scores are O(30) max), denominator via ones-matmul,
causal masking by static tile skipping + mask multiply. The q_b/absorb
path runs lazily per head inside the attention loop to bound SBUF.
"""

import numpy as np
import ml_dtypes

bfloat16 = ml_dtypes.bfloat16

T = 2048
H = 2048
NH = 16
QLR = 1536
KVLR = 512
DN = 128
DR = 64
DV = 128
INTER = 10944
NCORES = 8
RPC = T // NCORES
NQT = RPC // 128
NTT = T // 128
NFC = H // 128
NRC = QLR // 128
NKV = KVLR // 128
NIT = 86
IPAD = NIT * 128
EPS = 1e-6
SCALE = (DN + DR) ** -0.5
THETA = 10000.0
QH = DN + DR               # 192 per-head q dim

_CACHE = {}


def _build_module():
    import os
    MAXPH = int(os.environ.get("KERNEL_MAXPH", "9"))
    import concourse.bass as bass
    import concourse.tile as tile
    from concourse import bacc, mybir

    f32 = mybir.dt.float32
    bf16 = mybir.dt.bfloat16
    AF = mybir.ActivationFunctionType
    ALU = mybir.AluOpType

    nc = bacc.Bacc("TRN2", target_bir_lowering=False, debug=False,
                   enable_asserts=False, num_devices=NCORES)

    def inp(name, shape, dt):
        return nc.dram_tensor(name, list(shape), dt, kind="ExternalInput").ap()

    # per-core inputs
    x_rows = inp("x_rows", [NQT, 128, H], f32)
    xTc = inp("xTc", [NFC, 128, RPC], bf16)
    cosqT = inp("cosqT", [64, RPC], f32)
    sinqT = inp("sinqT", [64, RPC], f32)
    masks = inp("masks", [NTT, 128, RPC], bf16)
    permT = inp("permT", [64, 64], bf16)
    # replicated inputs
    xstat = inp("xstat", [NTT, 128, H], bf16)
    xT_blk = inp("xT_blk", [NTT, 128, NFC, 128], bf16)
    qa_blk = inp("qa_blk", [NFC, 128, QLR], bf16)
    qb_blk = inp("qb_blk", [NH, NRC, 128, QH], bf16)
    kva_blk = inp("kva_blk", [NFC, 128, KVLR + DR], bf16)
    wuk = inp("wuk", [NH, 128, NKV, 128], bf16)
    wuv = inp("wuv", [NH, 128, NKV, DV], bf16)
    ow_blk = inp("ow_blk", [NH, 128, H], bf16)
    gu_blk = inp("gu_blk", [2, NIT, 128, NFC, 128], bf16)
    dw_blk = inp("dw_blk", [NIT, 128, H], bf16)
    cosk = inp("cosk", [128, NTT, DR // 2], f32)
    sink = inp("sink", [128, NTT, DR // 2], f32)
    eye = inp("eye", [128, 128], bf16)
    ones = inp("ones", [128, 1], bf16)

    out_rows = nc.dram_tensor("out_rows", [NQT, 128, H], f32,
                              kind="ExternalOutput").ap()

    from contextlib import ExitStack
    with tile.TileContext(nc) as tc, ExitStack() as ctx:
        persist = ctx.enter_context(tc.tile_pool(name="persist", bufs=1))

        def pt(shape, dt, tag):
            return persist.tile(list(shape), dt, tag=tag, name=tag)

        eps_sb = pt([128, 1], f32, "eps")
        nc.vector.memset(eps_sb[:], EPS)
        eye_sb = pt([128, 128], bf16, "eye")
        nc.sync.dma_start(out=eye_sb[:], in_=eye[:])
        ones_sb = pt([128, 1], bf16, "ones")
        nc.sync.dma_start(out=ones_sb[:], in_=ones[:])
        x_rows_sb = pt([128, NQT, H], f32, "x_rows")
        for qt in range(NQT):
            nc.sync.dma_start(out=x_rows_sb[:, qt, :], in_=x_rows[qt])

        rstd_all = pt([128, NTT], f32, "rstd_all")
        s_ck = pt([128, NTT], f32, "s_ck")
        c_hat = pt([128, NTT, KVLR], bf16, "c_hat")
        kT_lat = pt([128, NKV, T], bf16, "kT_lat")
        kT_rope = pt([64, T], bf16, "kT_rope")
        qcT = pt([128, NRC, RPC], bf16, "qcT")
        o_vT = pt([128, NH, RPC], bf16, "o_vT")
        hnT = pt([128, NFC, RPC], bf16, "hnT")
        act_all = pt([128, NIT, RPC], bf16, "act_all")

        # ========== phases 0+1 interleaved: kv path + q_a ==========
        # Phase 1's GEMM is emitted between phase 0's kv GEMM and the kT
        # transposes so the PE stays busy while the kv norm/rope chain runs
        # on Vector/Scalar.
        with tc.tile_pool(name="p0s", bufs=1) as p0s, \
             tc.tile_pool(name="p0d", bufs=1) as p0d:
            cosk_sb = p0s.tile([128, NTT, DR // 2], f32, name="cosk_sb")
            nc.sync.dma_start(out=cosk_sb[:], in_=cosk[:])
            sink_sb = p0s.tile([128, NTT, DR // 2], f32, name="sink_sb")
            nc.sync.dma_start(out=sink_sb[:], in_=sink[:])
            ssq_all = p0s.tile([128, NTT], f32, name="ssq_all")
            ssq_kv = p0s.tile([128, NTT], f32, name="ssq_kv")
            c_raw = p0s.tile([128, NTT, KVLR + DR], bf16, name="c_raw")
            with tc.tile_pool(name="p0a", bufs=2) as p0a, \
                 tc.tile_pool(name="p0w", bufs=NFC) as p0w, \
                 tc.tile_pool(name="p0ps", bufs=2, space="PSUM") as p0ps:
                for tt in range(NTT):
                    xs = p0a.tile([128, H], bf16, tag="xs", name="xs")
                    nc.gpsimd.dma_start(out=xs[:], in_=xstat[tt])
                    scrap = p0d.tile([128, H], bf16, tag="scrap", name="scrap")
                    nc.vector.scalar_tensor_tensor(
                        scrap[:], xs[:], 1.0, xs[:], ALU.bypass, ALU.mult,
                        accum_out=ssq_all[:, tt:tt + 1])
                nc.scalar.activation(rstd_all[:], ssq_all[:], AF.Ln,
                                     bias=eps_sb[:], scale=1.0 / H)
                nc.scalar.activation(rstd_all[:], rstd_all[:], AF.Exp,
                                     scale=-0.5)

                kvw = []
                for fc in range(NFC):
                    w = p0w.tile([128, KVLR + DR], bf16, tag="kvw", name="kvw")
                    nc.sync.dma_start(out=w[:], in_=kva_blk[fc])
                    kvw.append(w)
                for tt in range(NTT):
                    xt = p0a.tile([128, NFC, 128], bf16, tag="xt", name="xt")
                    nc.gpsimd.dma_start(out=xt[:], in_=xT_blk[tt])
                    ps = p0ps.tile([128, KVLR + DR], f32, tag="kvps",
                                   name="kvps")
                    for fc in range(NFC):
                        nc.tensor.matmul(ps[:, 0:512], xt[:, fc, :],
                                         kvw[fc][:, 0:512],
                                         start=(fc == 0), stop=(fc == NFC - 1))
                        nc.tensor.matmul(ps[:, 512:576], xt[:, fc, :],
                                         kvw[fc][:, 512:576],
                                         start=(fc == 0), stop=(fc == NFC - 1))
                    scr2 = p0d.tile([128, KVLR], bf16, tag="scr2", name="scr2")
                    nc.scalar.activation(scr2[:], ps[:, 0:512], AF.Square,
                                         accum_out=ssq_kv[:, tt:tt + 1])
                    nc.vector.tensor_copy(c_raw[:, tt, :], ps[:])
            t1 = p0s.tile([128, NTT], f32, name="t1")
            nc.vector.tensor_mul(t1[:], rstd_all[:], rstd_all[:])
            nc.vector.tensor_mul(t1[:], t1[:], ssq_kv[:])
            nc.scalar.activation(t1[:], t1[:], AF.Ln, bias=eps_sb[:],
                                 scale=1.0 / KVLR)
            nc.scalar.activation(t1[:], t1[:], AF.Exp, scale=-0.5)
            nc.vector.tensor_mul(s_ck[:], rstd_all[:], t1[:])
            for tt in range(NTT):
                nc.vector.tensor_scalar_mul(c_hat[:, tt, :], c_raw[:, tt, 0:512],
                                            s_ck[:, tt:tt + 1])
            kr = p0s.tile([128, NTT, DR], bf16, name="kr")
            krf = p0s.tile([128, NTT, DR], bf16, name="krf")
            for tt in range(NTT):
                nc.vector.tensor_scalar_mul(kr[:, tt, :], c_raw[:, tt, 512:576],
                                            rstd_all[:, tt:tt + 1])
            x1 = kr[:, :, 0:DR:2]
            x2 = kr[:, :, 1:DR:2]
            ta = p0s.tile([128, NTT, DR // 2], f32, name="ta")
            tb = p0s.tile([128, NTT, DR // 2], f32, name="tb")
            nc.vector.tensor_mul(ta[:], x1, cosk_sb[:])
            nc.vector.tensor_mul(tb[:], x2, sink_sb[:])
            nc.vector.tensor_sub(krf[:, :, 0:DR:2], ta[:], tb[:])
            nc.vector.tensor_mul(ta[:], x2, cosk_sb[:])
            nc.vector.tensor_mul(tb[:], x1, sink_sb[:])
            nc.vector.tensor_add(krf[:, :, 1:DR:2], ta[:], tb[:])

            # ---- phase 1 (q_a -> qcT) emitted here: its GEMM keeps the PE
            # busy while the kv norm/rope chain above runs on Vector/Scalar
            if MAXPH >= 1:
                with tc.tile_pool(name="p1", bufs=3) as p1, \
                     tc.tile_pool(name="p1s", bufs=1) as p1s, \
                     tc.tile_pool(name="p1d", bufs=1) as p1d, \
                     tc.tile_pool(name="p1ps", bufs=2, space="PSUM") as p1ps, \
                     tc.tile_pool(name="p1tp", bufs=2, space="PSUM") as p1tp:
                    rstd_rows = p1s.tile([128, NQT], f32, name="rstd_rows")
                    ssq_r = p1s.tile([128, NQT], f32, name="ssq_r")
                    for qt in range(NQT):
                        scrap = p1d.tile([128, H], bf16, tag="scrapq",
                                         name="scrapq")
                        nc.vector.scalar_tensor_tensor(
                            scrap[:], x_rows_sb[:, qt, :], 1.0,
                            x_rows_sb[:, qt, :],
                            ALU.bypass, ALU.mult, accum_out=ssq_r[:, qt:qt + 1])
                    nc.scalar.activation(rstd_rows[:], ssq_r[:], AF.Ln,
                                         bias=eps_sb[:], scale=1.0 / H)
                    nc.scalar.activation(rstd_rows[:], rstd_rows[:], AF.Exp,
                                         scale=-0.5)

                    xTc_sb = p1s.tile([128, NFC, RPC], bf16, name="xTc_sb")
                    for fc in range(NFC):
                        nc.sync.dma_start(out=xTc_sb[:, fc, :], in_=xTc[fc])
                    qa_ps = [p1ps.tile([128, QLR], f32, tag="mm", name="mm")
                             for _ in range(NQT)]
                    for fc in range(NFC):
                        qaw = p1.tile([128, QLR], bf16, tag="qaw", name="qaw")
                        nc.sync.dma_start(out=qaw[:], in_=qa_blk[fc])
                        for qt in range(NQT):
                            for nn in range(QLR // 512):
                                nc.tensor.matmul(
                                    qa_ps[qt][:, nn * 512:(nn + 1) * 512],
                                    xTc_sb[:, fc, qt * 128:(qt + 1) * 128],
                                    qaw[:, nn * 512:(nn + 1) * 512],
                                    start=(fc == 0), stop=(fc == NFC - 1))
                    qc = p1s.tile([128, NQT, QLR], bf16, name="qc")
                    ssq_q = p1s.tile([128, NQT], f32, name="ssq_q")
                    for qt in range(NQT):
                        scr = p1d.tile([128, QLR], bf16, tag="scrq2",
                                       name="scrq2")
                        nc.scalar.activation(scr[:], qa_ps[qt][:], AF.Square,
                                             accum_out=ssq_q[:, qt:qt + 1])
                    sq = p1s.tile([128, NQT], f32, name="sq")
                    nc.vector.tensor_mul(sq[:], rstd_rows[:], rstd_rows[:])
                    nc.vector.tensor_mul(sq[:], sq[:], ssq_q[:])
                    nc.scalar.activation(sq[:], sq[:], AF.Ln, bias=eps_sb[:],
                                         scale=1.0 / QLR)
                    nc.scalar.activation(sq[:], sq[:], AF.Exp, scale=-0.5)
                    nc.vector.tensor_mul(sq[:], rstd_rows[:], sq[:])
                    for qt in range(NQT):
                        nc.vector.tensor_scalar_mul(qc[:, qt, :], qa_ps[qt][:],
                                                    sq[:, qt:qt + 1])
                    for qt in range(NQT):
                        for rc in range(NRC):
                            tp = p1tp.tile([128, 128], bf16, tag="tp",
                                           name="tp")
                            nc.tensor.transpose(
                                tp[:], qc[:, qt, rc * 128:(rc + 1) * 128],
                                eye_sb[:])
                            nc.any.tensor_copy(
                                qcT[:, rc, qt * 128:(qt + 1) * 128], tp[:])

            # ---- kT transposes after phase 1: PE-ordered behind it, and
            # phase 2's first q_b matmuls only need qcT anyway
            with tc.tile_pool(name="p0tp", bufs=2, space="PSUM") as p0tp:
                for tt in range(NTT):
                    for rc in range(NKV):
                        tp = p0tp.tile([128, 128], bf16, tag="tp", name="tp")
                        nc.tensor.transpose(
                            tp[:], c_hat[:, tt, rc * 128:(rc + 1) * 128],
                            eye_sb[:])
                        nc.any.tensor_copy(
                            kT_lat[:, rc, tt * 128:(tt + 1) * 128], tp[:])
                    tp = p0tp.tile([128, 128], bf16, tag="tp", name="tp")
                    nc.tensor.transpose(tp[0:64, :], krf[:, tt, :], eye_sb[:])
                    nc.any.tensor_copy(kT_rope[:, tt * 128:(tt + 1) * 128],
                                       tp[0:64, :])

        # ============ phase 2: head-pair q_b + attention ============
        # Heads processed in pairs: the kT_lat/kT_rope/c_hat stationaries are
        # shared across heads, so a 3D moving AP [128, 2, N] doubles the free
        # dim per LDWEIGHTS and halves the LDW count (the prior bottleneck).
        if MAXPH >= 2:
            with tc.tile_pool(name="p2", bufs=6) as p2, \
                 tc.tile_pool(name="p2s", bufs=1) as p2s, \
                 tc.tile_pool(name="p2d", bufs=2) as p2d, \
                 tc.tile_pool(name="p2e", bufs=6) as p2e, \
                 tc.tile_pool(name="p2r", bufs=2) as p2r, \
                 tc.tile_pool(name="pQ", bufs=1, space="PSUM") as pQ, \
                 tc.tile_pool(name="psp", bufs=2, space="PSUM") as psp, \
                 tc.tile_pool(name="pO", bufs=4, space="PSUM") as pO, \
                 tc.tile_pool(name="pD", bufs=1, space="PSUM") as pD:
                masks_sb = p2s.tile([128, NTT, RPC], bf16, name="masks_sb")
                for kt in range(NTT):
                    nc.scalar.dma_start(out=masks_sb[:, kt, :], in_=masks[kt])
                cosqT_sb = p2s.tile([64, RPC], f32, name="cosqT_sb")
                nc.sync.dma_start(out=cosqT_sb[:], in_=cosqT[:])
                sinqT_sb = p2s.tile([64, RPC], f32, name="sinqT_sb")
                nc.sync.dma_start(out=sinqT_sb[:], in_=sinqT[:])
                permT_sb = p2s.tile([64, 64], bf16, name="permT_sb")
                nc.sync.dma_start(out=permT_sb[:], in_=permT[:])

                for hp in range(NH // 2):
                    qT2 = p2d.tile([128, NKV, 2, RPC], bf16, tag="qT2",
                                   name="qT2")
                    qrT2 = p2d.tile([64, 2, RPC], bf16, tag="qrT2", name="qrT2")
                    wuv_hh = []
                    for hh in range(2):
                        h = 2 * hp + hh
                        qbw = p2.tile([128, NRC, QH], bf16, tag="qbw",
                                      name="qbw")
                        for rc in range(NRC):
                            nc.scalar.dma_start(out=qbw[:, rc, :],
                                                in_=qb_blk[h, rc])
                        wuk_h = p2.tile([128, NKV, 128], bf16, tag="wuk_h",
                                        name="wuk_h")
                        nc.scalar.dma_start(out=wuk_h[:], in_=wuk[h])
                        wuv_h = p2.tile([128, NKV, DV], bf16, tag="wuv_h",
                                        name="wuv_h")
                        nc.scalar.dma_start(out=wuv_h[:], in_=wuv[h])
                        wuv_hh.append(wuv_h)

                        # q_b transposed: nope [dn, tok] + rope [dr, tok]
                        # share one PSUM bank; only the first MM carries start
                        qnr_ps = pQ.tile([128, 512], f32, tag="qnr", name="qnr")
                        for rc in range(NRC):
                            nc.tensor.matmul(
                                qnr_ps[:, 0:RPC], qbw[:, rc, 0:DN],
                                qcT[:, rc, :],
                                start=(rc == 0), stop=(rc == NRC - 1),
                                skip_group_check=True)
                            nc.tensor.matmul(
                                qnr_ps[0:DR, RPC:2 * RPC], qbw[:, rc, DN:QH],
                                qcT[:, rc, :],
                                start=False, stop=(rc == NRC - 1),
                                skip_group_check=True)
                        qnT = p2d.tile([128, RPC], bf16, tag="qnT", name="qnT")
                        nc.vector.tensor_copy(qnT[:], qnr_ps[:, 0:RPC])
                        # rope via signed pair-swap perm matmul: C*x + P(S*x)
                        tS = p2d.tile([64, RPC], bf16, tag="tS", name="tS")
                        nc.vector.tensor_mul(tS[:], qnr_ps[0:DR, RPC:2 * RPC],
                                             sinqT_sb[:])
                        tC = p2d.tile([64, RPC], f32, tag="tC", name="tC")
                        nc.vector.tensor_mul(tC[:], qnr_ps[0:DR, RPC:2 * RPC],
                                             cosqT_sb[:])
                        pr_ps = psp.tile([128, 2 * RPC], f32, tag="sp",
                                         name="prps")
                        nc.tensor.matmul(pr_ps[0:DR, 0:RPC], permT_sb[:], tS[:],
                                         start=True, stop=True)
                        nc.vector.tensor_add(qrT2[:, hh, :], tC[:],
                                             pr_ps[0:DR, 0:RPC])
                        # absorb w_uk into q
                        for rc in range(NKV):
                            lp = psp.tile([128, 2 * RPC], f32, tag="sp",
                                          name="lp")
                            nc.tensor.matmul(lp[:, 0:RPC], wuk_h[:, rc, :],
                                             qnT[:], start=True, stop=True)
                            nc.vector.tensor_copy(qT2[:, rc, hh, :],
                                                  lp[:, 0:RPC])

                    olT_ps = [pO.tile([128, 2, RPC], f32, tag="olT", name="olT")
                              for _ in range(NKV)]
                    den_ps = pD.tile([1, 2, RPC], f32, tag="den", name="den")
                    for kt in range(NTT):
                        q0 = 16 * kt
                        sp3 = psp.tile([128, 2, RPC], f32, tag="sp", name="sp")
                        for dc in range(NKV):
                            nc.tensor.matmul(
                                sp3[:, :, q0:],
                                kT_lat[:, dc, kt * 128:(kt + 1) * 128],
                                qT2[:, dc, :, q0:],
                                start=(dc == 0), stop=False,
                                skip_group_check=True)
                        nc.tensor.matmul(
                            sp3[:, :, q0:],
                            kT_rope[:, kt * 128:(kt + 1) * 128],
                            qrT2[:, :, q0:],
                            start=False, stop=True, skip_group_check=True)
                        eT = p2e.tile([128, 2, RPC], bf16, tag="eT", name="eT")
                        nc.scalar.activation(eT[:, :, q0:], sp3[:, :, q0:],
                                             AF.Exp)
                        for hh in range(2):
                            nc.vector.tensor_mul(eT[:, hh, q0:], eT[:, hh, q0:],
                                                 masks_sb[:, kt, q0:])
                        for rc in range(NKV):
                            nc.tensor.matmul(
                                olT_ps[rc][:, :, q0:],
                                c_hat[:, kt, rc * 128:(rc + 1) * 128],
                                eT[:, :, q0:],
                                start=(kt == 0), stop=(kt == NTT - 1),
                                skip_group_check=True)
                        nc.tensor.matmul(
                            den_ps[0:1, :, q0:], ones_sb[:], eT[:, :, q0:],
                            start=(kt == 0), stop=(kt == NTT - 1),
                            skip_group_check=True)
                    # normalize: reciprocal row, broadcast, scale o_v columns
                    den_sb = p2r.tile([1, 2, RPC], f32, tag="den_sb",
                                      name="den_sb")
                    nc.vector.tensor_copy(den_sb[:], den_ps[:])
                    rinv = p2r.tile([1, 2, RPC], f32, tag="rinv", name="rinv")
                    nc.vector.reciprocal(rinv[:], den_sb[:])
                    rb = p2r.tile([128, 2, RPC], f32, tag="rb", name="rb")
                    nc.gpsimd.partition_broadcast(rb[:], rinv[0:1, :, :])
                    olT_sb = p2d.tile([128, NKV, 2, RPC], bf16, tag="olT_sb",
                                      name="olT_sb")
                    for rc in range(NKV):
                        nc.vector.tensor_copy(olT_sb[:, rc, :, :],
                                              olT_ps[rc][:])
                    for hh in range(2):
                        ovp = psp.tile([128, 2 * RPC], f32, tag="sp", name="ovp")
                        for rc in range(NKV):
                            nc.tensor.matmul(
                                ovp[:, 0:RPC], wuv_hh[hh][:, rc, :],
                                olT_sb[:, rc, hh, :],
                                start=(rc == 0), stop=(rc == NKV - 1))
                        nc.vector.tensor_mul(o_vT[:, 2 * hp + hh, :],
                                             ovp[:, 0:RPC], rb[:, hh, :])


        # =================== phase 3: o_proj + residual + post-norm ===========
        if MAXPH >= 3:
            with tc.tile_pool(name="p3", bufs=3) as p3, \
                 tc.tile_pool(name="p3s", bufs=1) as p3s, \
                 tc.tile_pool(name="p3d", bufs=2) as p3d:
                hn = p3s.tile([128, NQT, H], bf16, name="hn")
                with tc.tile_pool(name="p3ps", bufs=2, space="PSUM") as p3ps:
                    op_ps = [p3ps.tile([128, H], f32, tag="opps", name="opps")
                             for _ in range(NQT)]
                    for hc in range(NH):
                        oww = p3.tile([128, H], bf16, tag="oww", name="oww")
                        nc.scalar.dma_start(out=oww[:], in_=ow_blk[hc])
                        for qt in range(NQT):
                            for nn in range(4):
                                nc.tensor.matmul(
                                    op_ps[qt][:, nn * 512:(nn + 1) * 512],
                                    o_vT[:, hc, qt * 128:(qt + 1) * 128],
                                    oww[:, nn * 512:(nn + 1) * 512],
                                    start=(hc == 0), stop=(hc == NH - 1))
                    ssq2 = p3s.tile([128, NQT], f32, name="ssq2")
                    for qt in range(NQT):
                        nc.vector.tensor_add(x_rows_sb[:, qt, :],
                                             x_rows_sb[:, qt, :], op_ps[qt][:])
                    for qt in range(NQT):
                        scr = p3d.tile([128, H], bf16, tag="scr3", name="scr3")
                        nc.vector.scalar_tensor_tensor(
                            scr[:], x_rows_sb[:, qt, :], 1.0, x_rows_sb[:, qt, :],
                            ALU.bypass, ALU.mult, accum_out=ssq2[:, qt:qt + 1])
                    nc.scalar.activation(ssq2[:], ssq2[:], AF.Ln, bias=eps_sb[:],
                                         scale=1.0 / H)
                    nc.scalar.activation(ssq2[:], ssq2[:], AF.Exp, scale=-0.5)
                    for qt in range(NQT):
                        nc.vector.tensor_scalar_mul(hn[:, qt, :],
                                                    x_rows_sb[:, qt, :],
                                                    ssq2[:, qt:qt + 1])
                for qt in range(NQT):
                    nc.sync.dma_start_transpose(
                        hnT[:, :, qt * 128:(qt + 1) * 128], hn[:, qt, :])


        # =================== phase 4: MLP ===================
        # dw tiles stream on the scalar-engine HWDGE ring (gw/uw own the sync
        # ring), with the first NPRE emitted inside the 4a loop so the down
        # weights prefetch while gate/up still computes.
        if MAXPH >= 4:
            NPRE = 6
            with tc.tile_pool(name="p4b", bufs=NPRE) as p4b, \
                 tc.tile_pool(name="p4s", bufs=2) as p4s:
                dw_tiles = {}

                def dw_fetch(it):
                    dw = p4b.tile([128, H], bf16, tag="dw", name="dw")
                    nc.scalar.dma_start(out=dw[:], in_=dw_blk[it])
                    dw_tiles[it] = dw

                with tc.tile_pool(name="p4", bufs=3) as p4, \
                     tc.tile_pool(name="p4ps", bufs=2, space="PSUM") as p4ps:
                    for it in range(NIT):
                        gw = p4.tile([128, NFC, 128], bf16, tag="gw", name="gw")
                        nc.sync.dma_start(out=gw[:], in_=gu_blk[0, it])
                        uw = p4.tile([128, NFC, 128], bf16, tag="uw", name="uw")
                        nc.sync.dma_start(out=uw[:], in_=gu_blk[1, it])
                        gp = p4ps.tile([128, RPC], f32, tag="gp", name="gp")
                        up = p4ps.tile([128, RPC], f32, tag="up", name="up")
                        for fc in range(NFC):
                            nc.tensor.matmul(gp[:], gw[:, fc, :], hnT[:, fc, :],
                                             start=(fc == 0),
                                             stop=(fc == NFC - 1))
                            nc.tensor.matmul(up[:], uw[:, fc, :], hnT[:, fc, :],
                                             start=(fc == 0),
                                             stop=(fc == NFC - 1))
                        gs = p4.tile([128, RPC], bf16, tag="gs", name="gs")
                        nc.scalar.activation(gs[:], gp[:], AF.Silu)
                        nc.vector.tensor_mul(act_all[:, it, :], gs[:], up[:])
                        if NIT - NPRE <= it < NIT:
                            dw_fetch(it - (NIT - NPRE))
                with tc.tile_pool(name="p4bps", bufs=2, space="PSUM") as p4bps:
                    o_ps = [p4bps.tile([128, H], f32, tag="ops", name="ops")
                            for _ in range(NQT)]
                    for it in range(NIT):
                        if it not in dw_tiles:
                            dw_fetch(it)
                        dw = dw_tiles.pop(it)
                        for qt in range(NQT):
                            for nn in range(4):
                                nc.tensor.matmul(
                                    o_ps[qt][:, nn * 512:(nn + 1) * 512],
                                    act_all[:, it, qt * 128:(qt + 1) * 128],
                                    dw[:, nn * 512:(nn + 1) * 512],
                                    start=(it == 0), stop=(it == NIT - 1))
                    for qt in range(NQT):
                        fin = p4s.tile([128, H], f32, tag="fin", name="fin")
                        nc.vector.tensor_add(fin[:], x_rows_sb[:, qt, :],
                                             o_ps[qt][:])
                        nc.sync.dma_start(out=out_rows[qt], in_=fin[:])

        if MAXPH < 4:
            with tc.tile_pool(name="pex", bufs=2) as pex:
                for qt in range(NQT):
                    fin = pex.tile([128, H], f32, tag="finx", name="finx")
                    nc.vector.tensor_copy(fin[:], x_rows_sb[:, qt, :])
                    nc.sync.dma_start(out=out_rows[qt], in_=fin[:])
    nc.compile()
    return nc


def _host_prep(inputs):
    f32 = np.float32
    bf = bfloat16
    x = np.asarray(inputs["hidden_states"], f32)
    pos = np.asarray(inputs["positions"]).astype(f32)

    lnw_in = np.asarray(inputs["input_ln_w"], f32)
    q_a_w = np.asarray(inputs["q_a_w"], f32) * lnw_in[:, None]
    kv_a_w = np.asarray(inputs["kv_a_w"], f32) * lnw_in[:, None]
    q_b_w = (np.asarray(inputs["q_b_w"], f32)
             * np.asarray(inputs["q_a_ln_w"], f32)[:, None]) * SCALE
    kvln = np.asarray(inputs["kv_a_ln_w"], f32)
    w_uk = np.asarray(inputs["w_uk"], f32) * kvln[:, None, None]
    w_uv = np.asarray(inputs["w_uv"], f32) * kvln[:, None, None]
    o_w = np.asarray(inputs["o_w"], f32)
    pln = np.asarray(inputs["post_ln_w"], f32)
    gate_w = np.asarray(inputs["gate_w"], f32) * pln[:, None]
    up_w = np.asarray(inputs["up_w"], f32) * pln[:, None]
    down_w = np.asarray(inputs["down_w"], f32)

    xT = np.ascontiguousarray(x.T)
    inv_freq = 1.0 / (THETA ** (np.arange(0, DR, 2, dtype=f32) / DR))
    ang = pos[:, None] * inv_freq
    cos_t = np.cos(ang).astype(f32)
    sin_t = np.sin(ang).astype(f32)

    gu = np.zeros((2, IPAD, H), f32)
    gu[0, :INTER] = gate_w.T
    gu[1, :INTER] = up_w.T

    rep = {
        "xstat": np.ascontiguousarray(x.reshape(NTT, 128, H).astype(bf)),
        "xT_blk": np.ascontiguousarray(
            xT.astype(bf).reshape(NFC, 128, NTT, 128).transpose(2, 1, 0, 3)),
        "qa_blk": np.ascontiguousarray(q_a_w.astype(bf).reshape(NFC, 128, QLR)),
        # qb_blk[h, rc, p, d] = q_b_w[rc*128+p, h*192+d]
        "qb_blk": np.ascontiguousarray(
            q_b_w.astype(bf).reshape(NRC, 128, NH, QH).transpose(2, 0, 1, 3)),
        "kva_blk": np.ascontiguousarray(
            kv_a_w.astype(bf).reshape(NFC, 128, KVLR + DR)),
        # wuk[h, d, rc, rr] = w_uk[rc*128+rr, h, d]
        "wuk": np.ascontiguousarray(
            w_uk.transpose(1, 2, 0).reshape(NH, 128, NKV, 128).astype(bf)),
        # wuv[h, p, rc, dv] = w_uv[rc*128+p, h, dv]
        "wuv": np.ascontiguousarray(
            w_uv.transpose(1, 0, 2).reshape(NH, NKV, 128, DV)
            .transpose(0, 2, 1, 3).astype(bf)),
        "ow_blk": np.ascontiguousarray(o_w.astype(bf).reshape(NH, 128, H)),
        "gu_blk": np.ascontiguousarray(
            gu.reshape(2, NIT, 128, NFC, 128).transpose(0, 1, 4, 3, 2)
            .astype(bf)),
        "dw_blk": np.ascontiguousarray(
            np.concatenate([down_w, np.zeros((IPAD - INTER, H), f32)], 0)
            .astype(bf).reshape(NIT, 128, H)),
        "cosk": np.ascontiguousarray(
            cos_t.reshape(NTT, 128, DR // 2).transpose(1, 0, 2)),
        "sink": np.ascontiguousarray(
            sin_t.reshape(NTT, 128, DR // 2).transpose(1, 0, 2)),
        "eye": np.eye(128, dtype=bf),
        "ones": np.ones((128, 1), bf),
    }
    # rope pair-swap permutation: out = M @ v; lhsT = M.T
    M = np.zeros((DR, DR), f32)
    for i in range(DR // 2):
        M[2 * i, 2 * i + 1] = -1.0
        M[2 * i + 1, 2 * i] = 1.0
    rep["permT"] = np.ascontiguousarray(M.T).astype(bf)

    per_core = []
    for c in range(NCORES):
        rows = np.arange(c, T, NCORES)
        m = dict(rep)
        m["x_rows"] = np.ascontiguousarray(x[rows].reshape(NQT, 128, H))
        m["xTc"] = np.ascontiguousarray(
            xT[:, rows].astype(bf).reshape(NFC, 128, RPC))
        # [64, RPC] rope tables, row d -> freq d//2
        m["cosqT"] = np.ascontiguousarray(
            np.repeat(cos_t[rows].T, 2, axis=0).astype(f32))
        m["sinqT"] = np.ascontiguousarray(
            np.repeat(sin_t[rows].T, 2, axis=0).astype(f32))
        mask = np.zeros((NTT, 128, RPC), f32)
        kpos = np.arange(128)
        for kt in range(NTT):
            gk = kt * 128 + kpos
            mask[kt] = (gk[:, None] <= rows[None, :]).astype(f32)
        m["masks"] = mask.astype(bf)
        per_core.append(m)
    return per_core


def kernel(**inputs):
    from concourse import bass_utils

    if "nc" not in _CACHE:
        _CACHE["nc"] = _build_module()
    nc = _CACHE["nc"]

    import os
    in_maps = _host_prep(inputs)
    trace = bool(os.environ.get("BASS_KERNEL_TRACE"))
    res = bass_utils.run_bass_kernel_spmd(nc, in_maps,
                                          core_ids=list(range(NCORES)),
                                          trace=trace)
    if trace and res.exec_time_ns is not None:
        print(f"HW exec time: {res.exec_time_ns} ns")
        _CACHE["last_result"] = res
    out = np.zeros((T, H), np.float32)
    for c in range(NCORES):
        rows = np.arange(c, T, NCORES)
        out[rows] = res.results[c]["out_rows"].reshape(RPC, H)
    return out



# revision 33
# speedup vs baseline: 1.0867x; 1.0005x over previous
"""DeepseekV3 decoder layer (MLA + SwiGLU MLP), T=2048 prefill, fp32 I/O.

Sharding: sequence-parallel striped — core c owns token rows c::8 (256 rows),
so all 8 cores run one identical SPMD program with balanced causal work; only
input data differs per core. The KV latent path (all 2048 tokens) is
replicated on every core; outputs are disjoint row sets concatenated on host.

Per core: row-major activations (per-token norm scales are per-partition),
bf16 matmul operands with fp32 PSUM accumulation, LN weights folded into
adjacent GEMMs on host, RoPE via host cos/sin tables, softmax without
max-subtraction (scores are O(30) max), denominator via ones-matmul,
causal masking by static tile skipping + mask multiply. The q_b/absorb
path runs lazily per head inside the attention loop to bound SBUF.
"""

import numpy as np
import ml_dtypes

bfloat16 = ml_dtypes.bfloat16

T = 2048
H = 2048
NH = 16
QLR = 1536
KVLR = 512
DN = 128
DR = 64
DV = 128
INTER = 10944
NCORES = 8
RPC = T // NCORES
NQT = RPC // 128
NTT = T // 128
NFC = H // 128
NRC = QLR // 128
NKV = KVLR // 128
NIT = 86
IPAD = NIT * 128
EPS = 1e-6
SCALE = (DN + DR) ** -0.5
THETA = 10000.0
QH = DN + DR               # 192 per-head q dim

_CACHE = {}


def _build_module():
    import os
    MAXPH = int(os.environ.get("KERNEL_MAXPH", "9"))
    import concourse.bass as bass
    import concourse.tile as tile
    from concourse import bacc, mybir

    f32 = mybir.dt.float32
    bf16 = mybir.dt.bfloat16
    AF = mybir.ActivationFunctionType
    ALU = mybir.AluOpType

    nc = bacc.Bacc("TRN2", target_bir_lowering=False, debug=False,
                   enable_asserts=False, num_devices=NCORES)

    def inp(name, shape, dt):
        return nc.dram_tensor(name, list(shape), dt, kind="ExternalInput").ap()

    # per-core inputs
    x_rows = inp("x_rows", [NQT, 128, H], f32)
    xTc = inp("xTc", [NFC, 128, RPC], bf16)
    cosqT = inp("cosqT", [64, RPC], f32)
    sinqT = inp("sinqT", [64, RPC], f32)
    masks = inp("masks", [NTT, 128, RPC], bf16)
    permT = inp("permT", [64, 64], bf16)
    # replicated inputs
    xstat = inp("xstat", [NTT, 128, H], bf16)
    xT_blk = inp("xT_blk", [NTT, 128, NFC, 128], bf16)
    qa_blk = inp("qa_blk", [NFC, 128, QLR], bf16)
    qb_blk = inp("qb_blk", [NH, NRC, 128, QH], bf16)
    kva_blk = inp("kva_blk", [NFC, 128, KVLR + DR], bf16)
    wuk = inp("wuk", [NH, 128, NKV, 128], bf16)
    wuv = inp("wuv", [NH, 128, NKV, DV], bf16)
    ow_blk = inp("ow_blk", [NH, 128, H], bf16)
    gu_blk = inp("gu_blk", [2, NIT, 128, NFC, 128], bf16)
    dw_blk = inp("dw_blk", [NIT, 128, H], bf16)
    cosk = inp("cosk", [128, NTT, DR // 2], f32)
    sink = inp("sink", [128, NTT, DR // 2], f32)
    eye = inp("eye", [128, 128], bf16)
    ones = inp("ones", [128, 1], bf16)

    out_rows = nc.dram_tensor("out_rows", [NQT, 128, H], f32,
                              kind="ExternalOutput").ap()

    from contextlib import ExitStack
    with tile.TileContext(nc) as tc, ExitStack() as ctx:
        persist = ctx.enter_context(tc.tile_pool(name="persist", bufs=1))

        def pt(shape, dt, tag):
            return persist.tile(list(shape), dt, tag=tag, name=tag)

        eps_sb = pt([128, 1], f32, "eps")
        nc.vector.memset(eps_sb[:], EPS)
        eye_sb = pt([128, 128], bf16, "eye")
        nc.sync.dma_start(out=eye_sb[:], in_=eye[:])
        ones_sb = pt([128, 1], bf16, "ones")
        nc.sync.dma_start(out=ones_sb[:], in_=ones[:])
        x_rows_sb = pt([128, NQT, H], f32, "x_rows")
        for qt in range(NQT):
            nc.sync.dma_start(out=x_rows_sb[:, qt, :], in_=x_rows[qt])

        rstd_all = pt([128, NTT], f32, "rstd_all")
        s_ck = pt([128, NTT], f32, "s_ck")
        c_hat = pt([128, NTT, KVLR], bf16, "c_hat")
        kT_lat = pt([128, NKV, T], bf16, "kT_lat")
        kT_rope = pt([64, T], bf16, "kT_rope")
        qcT = pt([128, NRC, RPC], bf16, "qcT")
        o_vT = pt([128, NH, RPC], bf16, "o_vT")
        hnT = pt([128, NFC, RPC], bf16, "hnT")
        act_all = pt([128, NIT, RPC], bf16, "act_all")

        # ========== phases 0+1 interleaved: kv path + q_a ==========
        # Phase 1's GEMM is emitted between phase 0's kv GEMM and the kT
        # transposes so the PE stays busy while the kv norm/rope chain runs
        # on Vector/Scalar.
        with tc.tile_pool(name="p0s", bufs=1) as p0s, \
             tc.tile_pool(name="p0d", bufs=1) as p0d:
            cosk_sb = p0s.tile([128, NTT, DR // 2], f32, name="cosk_sb")
            nc.sync.dma_start(out=cosk_sb[:], in_=cosk[:])
            sink_sb = p0s.tile([128, NTT, DR // 2], f32, name="sink_sb")
            nc.sync.dma_start(out=sink_sb[:], in_=sink[:])
            ssq_all = p0s.tile([128, NTT], f32, name="ssq_all")
            ssq_kv = p0s.tile([128, NTT], f32, name="ssq_kv")
            c_raw = p0s.tile([128, NTT, KVLR + DR], bf16, name="c_raw")
            with tc.tile_pool(name="p0a", bufs=2) as p0a, \
                 tc.tile_pool(name="p0w", bufs=NFC) as p0w, \
                 tc.tile_pool(name="p0ps", bufs=2, space="PSUM") as p0ps:
                for tt in range(NTT):
                    xs = p0a.tile([128, H], bf16, tag="xs", name="xs")
                    nc.gpsimd.dma_start(out=xs[:], in_=xstat[tt])
                    scrap = p0d.tile([128, H], bf16, tag="scrap", name="scrap")
                    nc.vector.scalar_tensor_tensor(
                        scrap[:], xs[:], 1.0, xs[:], ALU.bypass, ALU.mult,
                        accum_out=ssq_all[:, tt:tt + 1])
                nc.scalar.activation(rstd_all[:], ssq_all[:], AF.Ln,
                                     bias=eps_sb[:], scale=1.0 / H)
                nc.scalar.activation(rstd_all[:], rstd_all[:], AF.Exp,
                                     scale=-0.5)

                kvw = []
                for fc in range(NFC):
                    w = p0w.tile([128, KVLR + DR], bf16, tag="kvw", name="kvw")
                    nc.sync.dma_start(out=w[:], in_=kva_blk[fc])
                    kvw.append(w)
                for tt in range(NTT):
                    xt = p0a.tile([128, NFC, 128], bf16, tag="xt", name="xt")
                    nc.gpsimd.dma_start(out=xt[:], in_=xT_blk[tt])
                    ps = p0ps.tile([128, KVLR + DR], f32, tag="kvps",
                                   name="kvps")
                    for fc in range(NFC):
                        nc.tensor.matmul(ps[:, 0:512], xt[:, fc, :],
                                         kvw[fc][:, 0:512],
                                         start=(fc == 0), stop=(fc == NFC - 1))
                        nc.tensor.matmul(ps[:, 512:576], xt[:, fc, :],
                                         kvw[fc][:, 512:576],
                                         start=(fc == 0), stop=(fc == NFC - 1))
                    scr2 = p0d.tile([128, KVLR], bf16, tag="scr2", name="scr2")
                    nc.scalar.activation(scr2[:], ps[:, 0:512], AF.Square,
                                         accum_out=ssq_kv[:, tt:tt + 1])
                    nc.vector.tensor_copy(c_raw[:, tt, :], ps[:])
            t1 = p0s.tile([128, NTT], f32, name="t1")
            nc.vector.tensor_mul(t1[:], rstd_all[:], rstd_all[:])
            nc.vector.tensor_mul(t1[:], t1[:], ssq_kv[:])
            nc.scalar.activation(t1[:], t1[:], AF.Ln, bias=eps_sb[:],
                                 scale=1.0 / KVLR)
            nc.scalar.activation(t1[:], t1[:], AF.Exp, scale=-0.5)
            nc.vector.tensor_mul(s_ck[:], rstd_all[:], t1[:])
            for tt in range(NTT):
                nc.vector.tensor_scalar_mul(c_hat[:, tt, :], c_raw[:, tt, 0:512],
                                            s_ck[:, tt:tt + 1])
            kr = p0s.tile([128, NTT, DR], bf16, name="kr")
            krf = p0s.tile([128, NTT, DR], bf16, name="krf")
            for tt in range(NTT):
                nc.vector.tensor_scalar_mul(kr[:, tt, :], c_raw[:, tt, 512:576],
                                            rstd_all[:, tt:tt + 1])
            x1 = kr[:, :, 0:DR:2]
            x2 = kr[:, :, 1:DR:2]
            ta = p0s.tile([128, NTT, DR // 2], f32, name="ta")
            tb = p0s.tile([128, NTT, DR // 2], f32, name="tb")
            nc.vector.tensor_mul(ta[:], x1, cosk_sb[:])
            nc.vector.tensor_mul(tb[:], x2, sink_sb[:])
            nc.vector.tensor_sub(krf[:, :, 0:DR:2], ta[:], tb[:])
            nc.vector.tensor_mul(ta[:], x2, cosk_sb[:])
            nc.vector.tensor_mul(tb[:], x1, sink_sb[:])
            nc.vector.tensor_add(krf[:, :, 1:DR:2], ta[:], tb[:])

            # ---- phase 1 (q_a -> qcT) emitted here: its GEMM keeps the PE
            # busy while the kv norm/rope chain above runs on Vector/Scalar
            if MAXPH >= 1:
                with tc.tile_pool(name="p1", bufs=3) as p1, \
                     tc.tile_pool(name="p1s", bufs=1) as p1s, \
                     tc.tile_pool(name="p1d", bufs=1) as p1d, \
                     tc.tile_pool(name="p1ps", bufs=2, space="PSUM") as p1ps, \
                     tc.tile_pool(name="p1tp", bufs=2, space="PSUM") as p1tp:
                    rstd_rows = p1s.tile([128, NQT], f32, name="rstd_rows")
                    ssq_r = p1s.tile([128, NQT], f32, name="ssq_r")
                    for qt in range(NQT):
                        scrap = p1d.tile([128, H], bf16, tag="scrapq",
                                         name="scrapq")
                        nc.vector.scalar_tensor_tensor(
                            scrap[:], x_rows_sb[:, qt, :], 1.0,
                            x_rows_sb[:, qt, :],
                            ALU.bypass, ALU.mult, accum_out=ssq_r[:, qt:qt + 1])
                    nc.scalar.activation(rstd_rows[:], ssq_r[:], AF.Ln,
                                         bias=eps_sb[:], scale=1.0 / H)
                    nc.scalar.activation(rstd_rows[:], rstd_rows[:], AF.Exp,
                                         scale=-0.5)

                    xTc_sb = p1s.tile([128, NFC, RPC], bf16, name="xTc_sb")
                    for fc in range(NFC):
                        nc.sync.dma_start(out=xTc_sb[:, fc, :], in_=xTc[fc])
                    qa_ps = [p1ps.tile([128, QLR], f32, tag="mm", name="mm")
                             for _ in range(NQT)]
                    for fc in range(NFC):
                        qaw = p1.tile([128, QLR], bf16, tag="qaw", name="qaw")
                        nc.sync.dma_start(out=qaw[:], in_=qa_blk[fc])
                        for qt in range(NQT):
                            for nn in range(QLR // 512):
                                nc.tensor.matmul(
                                    qa_ps[qt][:, nn * 512:(nn + 1) * 512],
                                    xTc_sb[:, fc, qt * 128:(qt + 1) * 128],
                                    qaw[:, nn * 512:(nn + 1) * 512],
                                    start=(fc == 0), stop=(fc == NFC - 1))
                    # kT transposes here: PE-ordered right after the
                    # q_a GEMM, covering the qc norm-chain latency
                    for tt in range(NTT):
                        for rc in range(NKV):
                            tp = p1tp.tile([128, 128], bf16, tag="tp",
                                           name="tp")
                            nc.tensor.transpose(
                                tp[:], c_hat[:, tt, rc * 128:(rc + 1) * 128],
                                eye_sb[:])
                            nc.any.tensor_copy(
                                kT_lat[:, rc, tt * 128:(tt + 1) * 128], tp[:])
                        tp = p1tp.tile([128, 128], bf16, tag="tp", name="tp")
                        nc.tensor.transpose(tp[0:64, :], krf[:, tt, :],
                                            eye_sb[:])
                        nc.any.tensor_copy(kT_rope[:, tt * 128:(tt + 1) * 128],
                                           tp[0:64, :])
                    qc = p1s.tile([128, NQT, QLR], bf16, name="qc")
                    ssq_q = p1s.tile([128, NQT], f32, name="ssq_q")
                    for qt in range(NQT):
                        scr = p1d.tile([128, QLR], bf16, tag="scrq2",
                                       name="scrq2")
                        nc.scalar.activation(scr[:], qa_ps[qt][:], AF.Square,
                                             accum_out=ssq_q[:, qt:qt + 1])
                    sq = p1s.tile([128, NQT], f32, name="sq")
                    nc.vector.tensor_mul(sq[:], rstd_rows[:], rstd_rows[:])
                    nc.vector.tensor_mul(sq[:], sq[:], ssq_q[:])
                    nc.scalar.activation(sq[:], sq[:], AF.Ln, bias=eps_sb[:],
                                         scale=1.0 / QLR)
                    nc.scalar.activation(sq[:], sq[:], AF.Exp, scale=-0.5)
                    nc.vector.tensor_mul(sq[:], rstd_rows[:], sq[:])
                    for qt in range(NQT):
                        nc.vector.tensor_scalar_mul(qc[:, qt, :], qa_ps[qt][:],
                                                    sq[:, qt:qt + 1])
                    for qt in range(NQT):
                        for rc in range(NRC):
                            tp = p1tp.tile([128, 128], bf16, tag="tp",
                                           name="tp")
                            nc.tensor.transpose(
                                tp[:], qc[:, qt, rc * 128:(rc + 1) * 128],
                                eye_sb[:])
                            nc.any.tensor_copy(
                                qcT[:, rc, qt * 128:(qt + 1) * 128], tp[:])

            if MAXPH < 1:
                with tc.tile_pool(name="p0tp", bufs=2, space="PSUM") as p0tp:
                    for tt in range(NTT):
                        for rc in range(NKV):
                            tp = p0tp.tile([128, 128], bf16, tag="tp",
                                           name="tp")
                            nc.tensor.transpose(
                                tp[:], c_hat[:, tt, rc * 128:(rc + 1) * 128],
                                eye_sb[:])
                            nc.any.tensor_copy(
                                kT_lat[:, rc, tt * 128:(tt + 1) * 128], tp[:])
                        tp = p0tp.tile([128, 128], bf16, tag="tp", name="tp")
                        nc.tensor.transpose(tp[0:64, :], krf[:, tt, :],
                                            eye_sb[:])
                        nc.any.tensor_copy(
                            kT_rope[:, tt * 128:(tt + 1) * 128], tp[0:64, :])

        # ============ phase 2: head-pair q_b + attention ============
        # Heads processed in pairs: the kT_lat/kT_rope/c_hat stationaries are
        # shared across heads, so a 3D moving AP [128, 2, N] doubles the free
        # dim per LDWEIGHTS and halves the LDW count (the prior bottleneck).
        if MAXPH >= 2:
            with tc.tile_pool(name="p2", bufs=6) as p2, \
                 tc.tile_pool(name="p2s", bufs=1) as p2s, \
                 tc.tile_pool(name="p2d", bufs=2) as p2d, \
                 tc.tile_pool(name="p2e", bufs=6) as p2e, \
                 tc.tile_pool(name="p2r", bufs=2) as p2r, \
                 tc.tile_pool(name="pQ", bufs=1, space="PSUM") as pQ, \
                 tc.tile_pool(name="psp", bufs=2, space="PSUM") as psp, \
                 tc.tile_pool(name="pO", bufs=4, space="PSUM") as pO, \
                 tc.tile_pool(name="pD", bufs=1, space="PSUM") as pD:
                masks_sb = p2s.tile([128, NTT, RPC], bf16, name="masks_sb")
                for kt in range(NTT):
                    nc.scalar.dma_start(out=masks_sb[:, kt, :], in_=masks[kt])
                cosqT_sb = p2s.tile([64, RPC], f32, name="cosqT_sb")
                nc.sync.dma_start(out=cosqT_sb[:], in_=cosqT[:])
                sinqT_sb = p2s.tile([64, RPC], f32, name="sinqT_sb")
                nc.sync.dma_start(out=sinqT_sb[:], in_=sinqT[:])
                permT_sb = p2s.tile([64, 64], bf16, name="permT_sb")
                nc.sync.dma_start(out=permT_sb[:], in_=permT[:])

                for hp in range(NH // 2):
                    qT2 = p2d.tile([128, NKV, 2, RPC], bf16, tag="qT2",
                                   name="qT2")
                    qrT2 = p2d.tile([64, 2, RPC], bf16, tag="qrT2", name="qrT2")
                    wuv_hh = []
                    for hh in range(2):
                        h = 2 * hp + hh
                        qbw = p2.tile([128, NRC, QH], bf16, tag="qbw",
                                      name="qbw")
                        for rc in range(NRC):
                            nc.scalar.dma_start(out=qbw[:, rc, :],
                                                in_=qb_blk[h, rc])
                        wuk_h = p2.tile([128, NKV, 128], bf16, tag="wuk_h",
                                        name="wuk_h")
                        nc.scalar.dma_start(out=wuk_h[:], in_=wuk[h])
                        wuv_h = p2.tile([128, NKV, DV], bf16, tag="wuv_h",
                                        name="wuv_h")
                        nc.scalar.dma_start(out=wuv_h[:], in_=wuv[h])
                        wuv_hh.append(wuv_h)

                        # q_b transposed: nope [dn, tok] + rope [dr, tok]
                        # share one PSUM bank; only the first MM carries start
                        qnr_ps = pQ.tile([128, 512], f32, tag="qnr", name="qnr")
                        for rc in range(NRC):
                            nc.tensor.matmul(
                                qnr_ps[:, 0:RPC], qbw[:, rc, 0:DN],
                                qcT[:, rc, :],
                                start=(rc == 0), stop=(rc == NRC - 1),
                                skip_group_check=True)
                            nc.tensor.matmul(
                                qnr_ps[0:DR, RPC:2 * RPC], qbw[:, rc, DN:QH],
                                qcT[:, rc, :],
                                start=False, stop=(rc == NRC - 1),
                                skip_group_check=True)
                        qnT = p2d.tile([128, RPC], bf16, tag="qnT", name="qnT")
                        nc.vector.tensor_copy(qnT[:], qnr_ps[:, 0:RPC])
                        # rope via signed pair-swap perm matmul: C*x + P(S*x)
                        tS = p2d.tile([64, RPC], bf16, tag="tS", name="tS")
                        nc.vector.tensor_mul(tS[:], qnr_ps[0:DR, RPC:2 * RPC],
                                             sinqT_sb[:])
                        tC = p2d.tile([64, RPC], f32, tag="tC", name="tC")
                        nc.vector.tensor_mul(tC[:], qnr_ps[0:DR, RPC:2 * RPC],
                                             cosqT_sb[:])
                        pr_ps = psp.tile([128, 2 * RPC], f32, tag="sp",
                                         name="prps")
                        nc.tensor.matmul(pr_ps[0:DR, 0:RPC], permT_sb[:], tS[:],
                                         start=True, stop=True)
                        nc.vector.tensor_add(qrT2[:, hh, :], tC[:],
                                             pr_ps[0:DR, 0:RPC])
                        # absorb w_uk into q
                        for rc in range(NKV):
                            lp = psp.tile([128, 2 * RPC], f32, tag="sp",
                                          name="lp")
                            nc.tensor.matmul(lp[:, 0:RPC], wuk_h[:, rc, :],
                                             qnT[:], start=True, stop=True)
                            nc.vector.tensor_copy(qT2[:, rc, hh, :],
                                                  lp[:, 0:RPC])

                    olT_ps = [pO.tile([128, 2, RPC], f32, tag="olT", name="olT")
                              for _ in range(NKV)]
                    den_ps = pD.tile([1, 2, RPC], f32, tag="den", name="den")
                    for kt in range(NTT):
                        q0 = 16 * kt
                        sp3 = psp.tile([128, 2, RPC], f32, tag="sp", name="sp")
                        for dc in range(NKV):
                            nc.tensor.matmul(
                                sp3[:, :, q0:],
                                kT_lat[:, dc, kt * 128:(kt + 1) * 128],
                                qT2[:, dc, :, q0:],
                                start=(dc == 0), stop=False,
                                skip_group_check=True)
                        nc.tensor.matmul(
                            sp3[:, :, q0:],
                            kT_rope[:, kt * 128:(kt + 1) * 128],
                            qrT2[:, :, q0:],
                            start=False, stop=True, skip_group_check=True)
                        eT = p2e.tile([128, 2, RPC], bf16, tag="eT", name="eT")
                        nc.scalar.activation(eT[:, :, q0:], sp3[:, :, q0:],
                                             AF.Exp)
                        for hh in range(2):
                            nc.vector.tensor_mul(eT[:, hh, q0:], eT[:, hh, q0:],
                                                 masks_sb[:, kt, q0:])
                        for rc in range(NKV):
                            nc.tensor.matmul(
                                olT_ps[rc][:, :, q0:],
                                c_hat[:, kt, rc * 128:(rc + 1) * 128],
                                eT[:, :, q0:],
                                start=(kt == 0), stop=(kt == NTT - 1),
                                skip_group_check=True)
                        nc.tensor.matmul(
                            den_ps[0:1, :, q0:], ones_sb[:], eT[:, :, q0:],
                            start=(kt == 0), stop=(kt == NTT - 1),
                            skip_group_check=True)
                    # normalize: reciprocal row, broadcast, scale o_v columns
                    den_sb = p2r.tile([1, 2, RPC], f32, tag="den_sb",
                                      name="den_sb")
                    nc.vector.tensor_copy(den_sb[:], den_ps[:])
                    rinv = p2r.tile([1, 2, RPC], f32, tag="rinv", name="rinv")
                    nc.vector.reciprocal(rinv[:], den_sb[:])
                    rb = p2r.tile([128, 2, RPC], f32, tag="rb", name="rb")
                    nc.gpsimd.partition_broadcast(rb[:], rinv[0:1, :, :])
                    olT_sb = p2d.tile([128, NKV, 2, RPC], bf16, tag="olT_sb",
                                      name="olT_sb")
                    for rc in range(NKV):
                        nc.vector.tensor_copy(olT_sb[:, rc, :, :],
                                              olT_ps[rc][:])
                    for hh in range(2):
                        ovp = psp.tile([128, 2 * RPC], f32, tag="sp", name="ovp")
                        for rc in range(NKV):
                            nc.tensor.matmul(
                                ovp[:, 0:RPC], wuv_hh[hh][:, rc, :],
                                olT_sb[:, rc, hh, :],
                                start=(rc == 0), stop=(rc == NKV - 1))
                        nc.vector.tensor_mul(o_vT[:, 2 * hp + hh, :],
                                             ovp[:, 0:RPC], rb[:, hh, :])


        # =================== phase 3: o_proj + residual + post-norm ===========
        if MAXPH >= 3:
            with tc.tile_pool(name="p3", bufs=3) as p3, \
                 tc.tile_pool(name="p3s", bufs=1) as p3s, \
                 tc.tile_pool(name="p3d", bufs=2) as p3d:
                hn = p3s.tile([128, NQT, H], bf16, name="hn")
                with tc.tile_pool(name="p3ps", bufs=2, space="PSUM") as p3ps:
                    op_ps = [p3ps.tile([128, H], f32, tag="opps", name="opps")
                             for _ in range(NQT)]
                    for hc in range(NH):
                        oww = p3.tile([128, H], bf16, tag="oww", name="oww")
                        nc.scalar.dma_start(out=oww[:], in_=ow_blk[hc])
                        for qt in range(NQT):
                            for nn in range(4):
                                nc.tensor.matmul(
                                    op_ps[qt][:, nn * 512:(nn + 1) * 512],
                                    o_vT[:, hc, qt * 128:(qt + 1) * 128],
                                    oww[:, nn * 512:(nn + 1) * 512],
                                    start=(hc == 0), stop=(hc == NH - 1))
                    ssq2 = p3s.tile([128, NQT], f32, name="ssq2")
                    for qt in range(NQT):
                        nc.vector.tensor_add(x_rows_sb[:, qt, :],
                                             x_rows_sb[:, qt, :], op_ps[qt][:])
                    for qt in range(NQT):
                        scr = p3d.tile([128, H], bf16, tag="scr3", name="scr3")
                        nc.vector.scalar_tensor_tensor(
                            scr[:], x_rows_sb[:, qt, :], 1.0, x_rows_sb[:, qt, :],
                            ALU.bypass, ALU.mult, accum_out=ssq2[:, qt:qt + 1])
                    nc.scalar.activation(ssq2[:], ssq2[:], AF.Ln, bias=eps_sb[:],
                                         scale=1.0 / H)
                    nc.scalar.activation(ssq2[:], ssq2[:], AF.Exp, scale=-0.5)
                    for qt in range(NQT):
                        nc.vector.tensor_scalar_mul(hn[:, qt, :],
                                                    x_rows_sb[:, qt, :],
                                                    ssq2[:, qt:qt + 1])
                for qt in range(NQT):
                    nc.sync.dma_start_transpose(
                        hnT[:, :, qt * 128:(qt + 1) * 128], hn[:, qt, :])


        # =================== phase 4: MLP ===================
        # dw tiles stream on the scalar-engine HWDGE ring (gw/uw own the sync
        # ring), with the first NPRE emitted inside the 4a loop so the down
        # weights prefetch while gate/up still computes.
        if MAXPH >= 4:
            NPRE = 6
            with tc.tile_pool(name="p4b", bufs=NPRE) as p4b, \
                 tc.tile_pool(name="p4s", bufs=2) as p4s:
                dw_tiles = {}

                def dw_fetch(it):
                    dw = p4b.tile([128, H], bf16, tag="dw", name="dw")
                    nc.scalar.dma_start(out=dw[:], in_=dw_blk[it])
                    dw_tiles[it] = dw

                with tc.tile_pool(name="p4", bufs=3) as p4, \
                     tc.tile_pool(name="p4ps", bufs=2, space="PSUM") as p4ps:
                    for it in range(NIT):
                        gw = p4.tile([128, NFC, 128], bf16, tag="gw", name="gw")
                        nc.sync.dma_start(out=gw[:], in_=gu_blk[0, it])
                        uw = p4.tile([128, NFC, 128], bf16, tag="uw", name="uw")
                        nc.sync.dma_start(out=uw[:], in_=gu_blk[1, it])
                        gp = p4ps.tile([128, RPC], f32, tag="gp", name="gp")
                        up = p4ps.tile([128, RPC], f32, tag="up", name="up")
                        for fc in range(NFC):
                            nc.tensor.matmul(gp[:], gw[:, fc, :], hnT[:, fc, :],
                                             start=(fc == 0),
                                             stop=(fc == NFC - 1))
                            nc.tensor.matmul(up[:], uw[:, fc, :], hnT[:, fc, :],
                                             start=(fc == 0),
                                             stop=(fc == NFC - 1))
                        gs = p4.tile([128, RPC], bf16, tag="gs", name="gs")
                        nc.scalar.activation(gs[:], gp[:], AF.Silu)
                        nc.vector.tensor_mul(act_all[:, it, :], gs[:], up[:])
                        if NIT - NPRE <= it < NIT:
                            dw_fetch(it - (NIT - NPRE))
                with tc.tile_pool(name="p4bps", bufs=2, space="PSUM") as p4bps:
                    o_ps = [p4bps.tile([128, H], f32, tag="ops", name="ops")
                            for _ in range(NQT)]
                    for it in range(NIT):
                        if it not in dw_tiles:
                            dw_fetch(it)
                        dw = dw_tiles.pop(it)
                        for qt in range(NQT):
                            for nn in range(4):
                                nc.tensor.matmul(
                                    o_ps[qt][:, nn * 512:(nn + 1) * 512],
                                    act_all[:, it, qt * 128:(qt + 1) * 128],
                                    dw[:, nn * 512:(nn + 1) * 512],
                                    start=(it == 0), stop=(it == NIT - 1))
                    for qt in range(NQT):
                        fin = p4s.tile([128, H], f32, tag="fin", name="fin")
                        nc.vector.tensor_add(fin[:], x_rows_sb[:, qt, :],
                                             o_ps[qt][:])
                        nc.sync.dma_start(out=out_rows[qt], in_=fin[:])

        if MAXPH < 4:
            with tc.tile_pool(name="pex", bufs=2) as pex:
                for qt in range(NQT):
                    fin = pex.tile([128, H], f32, tag="finx", name="finx")
                    nc.vector.tensor_copy(fin[:], x_rows_sb[:, qt, :])
                    nc.sync.dma_start(out=out_rows[qt], in_=fin[:])
    nc.compile()
    return nc


def _host_prep(inputs):
    f32 = np.float32
    bf = bfloat16
    x = np.asarray(inputs["hidden_states"], f32)
    pos = np.asarray(inputs["positions"]).astype(f32)

    lnw_in = np.asarray(inputs["input_ln_w"], f32)
    q_a_w = np.asarray(inputs["q_a_w"], f32) * lnw_in[:, None]
    kv_a_w = np.asarray(inputs["kv_a_w"], f32) * lnw_in[:, None]
    q_b_w = (np.asarray(inputs["q_b_w"], f32)
             * np.asarray(inputs["q_a_ln_w"], f32)[:, None]) * SCALE
    kvln = np.asarray(inputs["kv_a_ln_w"], f32)
    w_uk = np.asarray(inputs["w_uk"], f32) * kvln[:, None, None]
    w_uv = np.asarray(inputs["w_uv"], f32) * kvln[:, None, None]
    o_w = np.asarray(inputs["o_w"], f32)
    pln = np.asarray(inputs["post_ln_w"], f32)
    gate_w = np.asarray(inputs["gate_w"], f32) * pln[:, None]
    up_w = np.asarray(inputs["up_w"], f32) * pln[:, None]
    down_w = np.asarray(inputs["down_w"], f32)

    xT = np.ascontiguousarray(x.T)
    inv_freq = 1.0 / (THETA ** (np.arange(0, DR, 2, dtype=f32) / DR))
    ang = pos[:, None] * inv_freq
    cos_t = np.cos(ang).astype(f32)
    sin_t = np.sin(ang).astype(f32)

    gu = np.zeros((2, IPAD, H), f32)
    gu[0, :INTER] = gate_w.T
    gu[1, :INTER] = up_w.T

    rep = {
        "xstat": np.ascontiguousarray(x.reshape(NTT, 128, H).astype(bf)),
        "xT_blk": np.ascontiguousarray(
            xT.astype(bf).reshape(NFC, 128, NTT, 128).transpose(2, 1, 0, 3)),
        "qa_blk": np.ascontiguousarray(q_a_w.astype(bf).reshape(NFC, 128, QLR)),
        # qb_blk[h, rc, p, d] = q_b_w[rc*128+p, h*192+d]
        "qb_blk": np.ascontiguousarray(
            q_b_w.astype(bf).reshape(NRC, 128, NH, QH).transpose(2, 0, 1, 3)),
        "kva_blk": np.ascontiguousarray(
            kv_a_w.astype(bf).reshape(NFC, 128, KVLR + DR)),
        # wuk[h, d, rc, rr] = w_uk[rc*128+rr, h, d]
        "wuk": np.ascontiguousarray(
            w_uk.transpose(1, 2, 0).reshape(NH, 128, NKV, 128).astype(bf)),
        # wuv[h, p, rc, dv] = w_uv[rc*128+p, h, dv]
        "wuv": np.ascontiguousarray(
            w_uv.transpose(1, 0, 2).reshape(NH, NKV, 128, DV)
            .transpose(0, 2, 1, 3).astype(bf)),
        "ow_blk": np.ascontiguousarray(o_w.astype(bf).reshape(NH, 128, H)),
        "gu_blk": np.ascontiguousarray(
            gu.reshape(2, NIT, 128, NFC, 128).transpose(0, 1, 4, 3, 2)
            .astype(bf)),
        "dw_blk": np.ascontiguousarray(
            np.concatenate([down_w, np.zeros((IPAD - INTER, H), f32)], 0)
            .astype(bf).reshape(NIT, 128, H)),
        "cosk": np.ascontiguousarray(
            cos_t.reshape(NTT, 128, DR // 2).transpose(1, 0, 2)),
        "sink": np.ascontiguousarray(
            sin_t.reshape(NTT, 128, DR // 2).transpose(1, 0, 2)),
        "eye": np.eye(128, dtype=bf),
        "ones": np.ones((128, 1), bf),
    }
    # rope pair-swap permutation: out = M @ v; lhsT = M.T
    M = np.zeros((DR, DR), f32)
    for i in range(DR // 2):
        M[2 * i, 2 * i + 1] = -1.0
        M[2 * i + 1, 2 * i] = 1.0
    rep["permT"] = np.ascontiguousarray(M.T).astype(bf)

    per_core = []
    for c in range(NCORES):
        rows = np.arange(c, T, NCORES)
        m = dict(rep)
        m["x_rows"] = np.ascontiguousarray(x[rows].reshape(NQT, 128, H))
        m["xTc"] = np.ascontiguousarray(
            xT[:, rows].astype(bf).reshape(NFC, 128, RPC))
        # [64, RPC] rope tables, row d -> freq d//2
        m["cosqT"] = np.ascontiguousarray(
            np.repeat(cos_t[rows].T, 2, axis=0).astype(f32))
        m["sinqT"] = np.ascontiguousarray(
            np.repeat(sin_t[rows].T, 2, axis=0).astype(f32))
        mask = np.zeros((NTT, 128, RPC), f32)
        kpos = np.arange(128)
        for kt in range(NTT):
            gk = kt * 128 + kpos
            mask[kt] = (gk[:, None] <= rows[None, :]).astype(f32)
        m["masks"] = mask.astype(bf)
        per_core.append(m)
    return per_core


def kernel(**inputs):
    from concourse import bass_utils

    if "nc" not in _CACHE:
        _CACHE["nc"] = _build_module()
    nc = _CACHE["nc"]

    import os
    in_maps = _host_prep(inputs)
    trace = bool(os.environ.get("BASS_KERNEL_TRACE"))
    res = bass_utils.run_bass_kernel_spmd(nc, in_maps,
                                          core_ids=list(range(NCORES)),
                                          trace=trace)
    if trace and res.exec_time_ns is not None:
        print(f"HW exec time: {res.exec_time_ns} ns")
        _CACHE["last_result"] = res
    out = np.zeros((T, H), np.float32)
    for c in range(NCORES):
        rows = np.arange(c, T, NCORES)
        out[rows] = res.results[c]["out_rows"].reshape(RPC, H)
    return out



# revision 34
# speedup vs baseline: 1.1213x; 1.0318x over previous
"""DeepseekV3 decoder layer (MLA + SwiGLU MLP), T=2048 prefill, fp32 I/O.

Sharding: sequence-parallel striped — core c owns token rows c::8 (256 rows),
so all 8 cores run one identical SPMD program with balanced causal work; only
input data differs per core. The KV latent path (all 2048 tokens) is
replicated on every core; outputs are disjoint row sets concatenated on host.

Per core: row-major activations (per-token norm scales are per-partition),
bf16 matmul operands with fp32 PSUM accumulation, LN weights folded into
adjacent GEMMs on host, RoPE via host cos/sin tables, softmax without
max-subtraction (scores are O(30) max), denominator via ones-matmul,
causal masking by static tile skipping + mask multiply. The q_b/absorb
path runs lazily per head inside the attention loop to bound SBUF.
"""

import numpy as np
import ml_dtypes

bfloat16 = ml_dtypes.bfloat16

T = 2048
H = 2048
NH = 16
QLR = 1536
KVLR = 512
DN = 128
DR = 64
DV = 128
INTER = 10944
NCORES = 8
RPC = T // NCORES
NQT = RPC // 128
NTT = T // 128
NFC = H // 128
NRC = QLR // 128
NKV = KVLR // 128
NIT = 86
IPAD = NIT * 128
EPS = 1e-6
SCALE = (DN + DR) ** -0.5
THETA = 10000.0
QH = DN + DR               # 192 per-head q dim

_CACHE = {}


def _build_module():
    import os
    MAXPH = int(os.environ.get("KERNEL_MAXPH", "9"))
    import concourse.bass as bass
    import concourse.tile as tile
    from concourse import bacc, mybir

    f32 = mybir.dt.float32
    bf16 = mybir.dt.bfloat16
    AF = mybir.ActivationFunctionType
    ALU = mybir.AluOpType

    nc = bacc.Bacc("TRN2", target_bir_lowering=False, debug=False,
                   enable_asserts=False, num_devices=NCORES)

    def inp(name, shape, dt):
        return nc.dram_tensor(name, list(shape), dt, kind="ExternalInput").ap()

    # per-core inputs
    x_rows = inp("x_rows", [NQT, 128, H], f32)
    xTc = inp("xTc", [NFC, 128, RPC], bf16)
    cosqT = inp("cosqT", [64, RPC], f32)
    sinqT = inp("sinqT", [64, RPC], f32)
    masks = inp("masks", [NTT, 128, RPC], bf16)
    permT = inp("permT", [64, 64], bf16)
    # replicated inputs
    xstat = inp("xstat", [NTT, 128, H], bf16)
    xT_blk = inp("xT_blk", [NTT, 128, NFC, 128], bf16)
    qa_blk = inp("qa_blk", [NFC, 128, QLR], bf16)
    qb_blk = inp("qb_blk", [NH, NRC, 128, QH], bf16)
    kva_blk = inp("kva_blk", [NFC, 128, KVLR + DR], bf16)
    wuk = inp("wuk", [NH, 128, NKV, 128], bf16)
    wuv = inp("wuv", [NH, 128, NKV, DV], bf16)
    ow_blk = inp("ow_blk", [NH, 128, H], bf16)
    gu_blk = inp("gu_blk", [2, NIT, 128, NFC, 128], bf16)
    dw_blk = inp("dw_blk", [NIT, 128, H], bf16)
    cosk = inp("cosk", [128, NTT, DR // 2], f32)
    sink = inp("sink", [128, NTT, DR // 2], f32)
    eye = inp("eye", [128, 128], bf16)
    ones = inp("ones", [128, 1], bf16)

    out_rows = nc.dram_tensor("out_rows", [NQT, 128, H], f32,
                              kind="ExternalOutput").ap()

    from contextlib import ExitStack
    with tile.TileContext(nc) as tc, ExitStack() as ctx:
        persist = ctx.enter_context(tc.tile_pool(name="persist", bufs=1))

        def pt(shape, dt, tag):
            return persist.tile(list(shape), dt, tag=tag, name=tag)

        eps_sb = pt([128, 1], f32, "eps")
        nc.vector.memset(eps_sb[:], EPS)
        eye_sb = pt([128, 128], bf16, "eye")
        nc.sync.dma_start(out=eye_sb[:], in_=eye[:])
        ones_sb = pt([128, 1], bf16, "ones")
        nc.sync.dma_start(out=ones_sb[:], in_=ones[:])
        x_rows_sb = pt([128, NQT, H], f32, "x_rows")
        for qt in range(NQT):
            nc.sync.dma_start(out=x_rows_sb[:, qt, :], in_=x_rows[qt])

        rstd_all = pt([128, NTT], f32, "rstd_all")
        s_ck = pt([128, NTT], f32, "s_ck")
        c_hat = pt([128, NTT, KVLR], bf16, "c_hat")
        kT_lat = pt([128, NKV, T], bf16, "kT_lat")
        kT_rope = pt([64, T], bf16, "kT_rope")
        qcT = pt([128, NRC, RPC], bf16, "qcT")
        o_vT = pt([128, NH, RPC], bf16, "o_vT")
        hnT = pt([128, NFC, RPC], bf16, "hnT")
        act_all = pt([128, NIT, RPC], bf16, "act_all")

        # ========== phases 0+1 interleaved: kv path + q_a ==========
        # Phase 1's GEMM is emitted between phase 0's kv GEMM and the kT
        # transposes so the PE stays busy while the kv norm/rope chain runs
        # on Vector/Scalar.
        with tc.tile_pool(name="p0s", bufs=1) as p0s, \
             tc.tile_pool(name="p0d", bufs=1) as p0d:
            cosk_sb = p0s.tile([128, NTT, DR // 2], f32, name="cosk_sb")
            nc.sync.dma_start(out=cosk_sb[:], in_=cosk[:])
            sink_sb = p0s.tile([128, NTT, DR // 2], f32, name="sink_sb")
            nc.sync.dma_start(out=sink_sb[:], in_=sink[:])
            ssq_all = p0s.tile([128, NTT], f32, name="ssq_all")
            ssq_kv = p0s.tile([128, NTT], f32, name="ssq_kv")
            c_raw = p0s.tile([128, NTT, KVLR + DR], bf16, name="c_raw")
            with tc.tile_pool(name="p0a", bufs=2) as p0a, \
                 tc.tile_pool(name="p0w", bufs=NFC) as p0w, \
                 tc.tile_pool(name="p0ps", bufs=2, space="PSUM") as p0ps:
                for tt in range(NTT):
                    xs = p0a.tile([128, H], bf16, tag="xs", name="xs")
                    nc.gpsimd.dma_start(out=xs[:], in_=xstat[tt])
                    scrap = p0d.tile([128, H], bf16, tag="scrap", name="scrap")
                    nc.vector.scalar_tensor_tensor(
                        scrap[:], xs[:], 1.0, xs[:], ALU.bypass, ALU.mult,
                        accum_out=ssq_all[:, tt:tt + 1])
                nc.scalar.activation(rstd_all[:], ssq_all[:], AF.Ln,
                                     bias=eps_sb[:], scale=1.0 / H)
                nc.scalar.activation(rstd_all[:], rstd_all[:], AF.Exp,
                                     scale=-0.5)

                kvw = []
                for fc in range(NFC):
                    w = p0w.tile([128, KVLR + DR], bf16, tag="kvw", name="kvw")
                    nc.sync.dma_start(out=w[:], in_=kva_blk[fc])
                    kvw.append(w)
                for tt in range(NTT):
                    xt = p0a.tile([128, NFC, 128], bf16, tag="xt", name="xt")
                    nc.gpsimd.dma_start(out=xt[:], in_=xT_blk[tt])
                    ps = p0ps.tile([128, KVLR + DR], f32, tag="kvps",
                                   name="kvps")
                    for fc in range(NFC):
                        nc.tensor.matmul(ps[:, 0:512], xt[:, fc, :],
                                         kvw[fc][:, 0:512],
                                         start=(fc == 0), stop=(fc == NFC - 1))
                        nc.tensor.matmul(ps[:, 512:576], xt[:, fc, :],
                                         kvw[fc][:, 512:576],
                                         start=(fc == 0), stop=(fc == NFC - 1))
                    scr2 = p0d.tile([128, KVLR], bf16, tag="scr2", name="scr2")
                    nc.scalar.activation(scr2[:], ps[:, 0:512], AF.Square,
                                         accum_out=ssq_kv[:, tt:tt + 1])
                    nc.vector.tensor_copy(c_raw[:, tt, :], ps[:])
            t1 = p0s.tile([128, NTT], f32, name="t1")
            nc.vector.tensor_mul(t1[:], rstd_all[:], rstd_all[:])
            nc.vector.tensor_mul(t1[:], t1[:], ssq_kv[:])
            nc.scalar.activation(t1[:], t1[:], AF.Ln, bias=eps_sb[:],
                                 scale=1.0 / KVLR)
            nc.scalar.activation(t1[:], t1[:], AF.Exp, scale=-0.5)
            nc.vector.tensor_mul(s_ck[:], rstd_all[:], t1[:])
            for tt in range(NTT):
                nc.vector.tensor_scalar_mul(c_hat[:, tt, :], c_raw[:, tt, 0:512],
                                            s_ck[:, tt:tt + 1])
            kr = p0s.tile([128, NTT, DR], bf16, name="kr")
            krf = p0s.tile([128, NTT, DR], bf16, name="krf")
            for tt in range(NTT):
                nc.vector.tensor_scalar_mul(kr[:, tt, :], c_raw[:, tt, 512:576],
                                            rstd_all[:, tt:tt + 1])
            x1 = kr[:, :, 0:DR:2]
            x2 = kr[:, :, 1:DR:2]
            ta = p0s.tile([128, NTT, DR // 2], f32, name="ta")
            tb = p0s.tile([128, NTT, DR // 2], f32, name="tb")
            nc.vector.tensor_mul(ta[:], x1, cosk_sb[:])
            nc.vector.tensor_mul(tb[:], x2, sink_sb[:])
            nc.vector.tensor_sub(krf[:, :, 0:DR:2], ta[:], tb[:])
            nc.vector.tensor_mul(ta[:], x2, cosk_sb[:])
            nc.vector.tensor_mul(tb[:], x1, sink_sb[:])
            nc.vector.tensor_add(krf[:, :, 1:DR:2], ta[:], tb[:])

            # ---- phase 1 (q_a -> qcT) emitted here: its GEMM keeps the PE
            # busy while the kv norm/rope chain above runs on Vector/Scalar
            if MAXPH >= 1:
                with tc.tile_pool(name="p1", bufs=3) as p1, \
                     tc.tile_pool(name="p1s", bufs=1) as p1s, \
                     tc.tile_pool(name="p1d", bufs=1) as p1d, \
                     tc.tile_pool(name="p1ps", bufs=2, space="PSUM") as p1ps, \
                     tc.tile_pool(name="p1tp", bufs=2, space="PSUM") as p1tp:
                    rstd_rows = p1s.tile([128, NQT], f32, name="rstd_rows")
                    ssq_r = p1s.tile([128, NQT], f32, name="ssq_r")
                    for qt in range(NQT):
                        scrap = p1d.tile([128, H], bf16, tag="scrapq",
                                         name="scrapq")
                        nc.vector.scalar_tensor_tensor(
                            scrap[:], x_rows_sb[:, qt, :], 1.0,
                            x_rows_sb[:, qt, :],
                            ALU.bypass, ALU.mult, accum_out=ssq_r[:, qt:qt + 1])
                    nc.scalar.activation(rstd_rows[:], ssq_r[:], AF.Ln,
                                         bias=eps_sb[:], scale=1.0 / H)
                    nc.scalar.activation(rstd_rows[:], rstd_rows[:], AF.Exp,
                                         scale=-0.5)

                    xTc_sb = p1s.tile([128, NFC, RPC], bf16, name="xTc_sb")
                    for fc in range(NFC):
                        nc.sync.dma_start(out=xTc_sb[:, fc, :], in_=xTc[fc])
                    qa_ps = [p1ps.tile([128, QLR], f32, tag="mm", name="mm")
                             for _ in range(NQT)]
                    for fc in range(NFC):
                        qaw = p1.tile([128, QLR], bf16, tag="qaw", name="qaw")
                        nc.sync.dma_start(out=qaw[:], in_=qa_blk[fc])
                        for qt in range(NQT):
                            for nn in range(QLR // 512):
                                nc.tensor.matmul(
                                    qa_ps[qt][:, nn * 512:(nn + 1) * 512],
                                    xTc_sb[:, fc, qt * 128:(qt + 1) * 128],
                                    qaw[:, nn * 512:(nn + 1) * 512],
                                    start=(fc == 0), stop=(fc == NFC - 1))
                    # kT transposes here: PE-ordered right after the
                    # q_a GEMM, covering the qc norm-chain latency
                    for tt in range(NTT):
                        for rc in range(NKV):
                            tp = p1tp.tile([128, 128], bf16, tag="tp",
                                           name="tp")
                            nc.tensor.transpose(
                                tp[:], c_hat[:, tt, rc * 128:(rc + 1) * 128],
                                eye_sb[:])
                            nc.any.tensor_copy(
                                kT_lat[:, rc, tt * 128:(tt + 1) * 128], tp[:])
                        tp = p1tp.tile([128, 128], bf16, tag="tp", name="tp")
                        nc.tensor.transpose(tp[0:64, :], krf[:, tt, :],
                                            eye_sb[:])
                        nc.any.tensor_copy(kT_rope[:, tt * 128:(tt + 1) * 128],
                                           tp[0:64, :])
                    qc = p1s.tile([128, NQT, QLR], bf16, name="qc")
                    ssq_q = p1s.tile([128, NQT], f32, name="ssq_q")
                    for qt in range(NQT):
                        scr = p1d.tile([128, QLR], bf16, tag="scrq2",
                                       name="scrq2")
                        nc.scalar.activation(scr[:], qa_ps[qt][:], AF.Square,
                                             accum_out=ssq_q[:, qt:qt + 1])
                    sq = p1s.tile([128, NQT], f32, name="sq")
                    nc.vector.tensor_mul(sq[:], rstd_rows[:], rstd_rows[:])
                    nc.vector.tensor_mul(sq[:], sq[:], ssq_q[:])
                    nc.scalar.activation(sq[:], sq[:], AF.Ln, bias=eps_sb[:],
                                         scale=1.0 / QLR)
                    nc.scalar.activation(sq[:], sq[:], AF.Exp, scale=-0.5)
                    nc.vector.tensor_mul(sq[:], rstd_rows[:], sq[:])
                    for qt in range(NQT):
                        nc.vector.tensor_scalar_mul(qc[:, qt, :], qa_ps[qt][:],
                                                    sq[:, qt:qt + 1])
                    for qt in range(NQT):
                        for rc in range(NRC):
                            tp = p1tp.tile([128, 128], bf16, tag="tp",
                                           name="tp")
                            nc.tensor.transpose(
                                tp[:], qc[:, qt, rc * 128:(rc + 1) * 128],
                                eye_sb[:])
                            nc.any.tensor_copy(
                                qcT[:, rc, qt * 128:(qt + 1) * 128], tp[:])

            if MAXPH < 1:
                with tc.tile_pool(name="p0tp", bufs=2, space="PSUM") as p0tp:
                    for tt in range(NTT):
                        for rc in range(NKV):
                            tp = p0tp.tile([128, 128], bf16, tag="tp",
                                           name="tp")
                            nc.tensor.transpose(
                                tp[:], c_hat[:, tt, rc * 128:(rc + 1) * 128],
                                eye_sb[:])
                            nc.any.tensor_copy(
                                kT_lat[:, rc, tt * 128:(tt + 1) * 128], tp[:])
                        tp = p0tp.tile([128, 128], bf16, tag="tp", name="tp")
                        nc.tensor.transpose(tp[0:64, :], krf[:, tt, :],
                                            eye_sb[:])
                        nc.any.tensor_copy(
                            kT_rope[:, tt * 128:(tt + 1) * 128], tp[0:64, :])

        # ============ phase 2: head-pair q_b + attention ============
        # Heads processed in pairs: the kT_lat/kT_rope/c_hat stationaries are
        # shared across heads, so a 3D moving AP [128, 2, N] doubles the free
        # dim per LDWEIGHTS and halves the LDW count (the prior bottleneck).
        if MAXPH >= 2:
            with tc.tile_pool(name="p2", bufs=6) as p2, \
                 tc.tile_pool(name="p2s", bufs=1) as p2s, \
                 tc.tile_pool(name="p2d", bufs=2) as p2d, \
                 tc.tile_pool(name="p2e", bufs=6) as p2e, \
                 tc.tile_pool(name="p2r", bufs=2) as p2r, \
                 tc.tile_pool(name="pQ", bufs=1, space="PSUM") as pQ, \
                 tc.tile_pool(name="psp", bufs=2, space="PSUM") as psp, \
                 tc.tile_pool(name="pO", bufs=4, space="PSUM") as pO, \
                 tc.tile_pool(name="pD", bufs=1, space="PSUM") as pD:
                masks_sb = p2s.tile([128, NTT, RPC], bf16, name="masks_sb")
                for kt in range(NTT):
                    nc.scalar.dma_start(out=masks_sb[:, kt, :], in_=masks[kt])
                cosqT_sb = p2s.tile([64, RPC], f32, name="cosqT_sb")
                nc.sync.dma_start(out=cosqT_sb[:], in_=cosqT[:])
                sinqT_sb = p2s.tile([64, RPC], f32, name="sinqT_sb")
                nc.sync.dma_start(out=sinqT_sb[:], in_=sinqT[:])
                permT_sb = p2s.tile([64, 64], bf16, name="permT_sb")
                nc.sync.dma_start(out=permT_sb[:], in_=permT[:])

                for hp in range(NH // 2):
                    qT2 = p2d.tile([128, NKV, 2, RPC], bf16, tag="qT2",
                                   name="qT2")
                    qrT2 = p2d.tile([64, 2, RPC], bf16, tag="qrT2", name="qrT2")
                    wuv_hh = []
                    wuk_hh = []
                    qnr_hh = []
                    # pass 1: both heads' q_b matmuls back-to-back on the PE
                    # (head b's accumulator borrows the idle den PSUM slot)
                    for hh in range(2):
                        h = 2 * hp + hh
                        qbw = p2.tile([128, NRC, QH], bf16, tag="qbw",
                                      name="qbw")
                        for rc in range(NRC):
                            nc.scalar.dma_start(out=qbw[:, rc, :],
                                                in_=qb_blk[h, rc])
                        wuk_h = p2.tile([128, NKV, 128], bf16, tag="wuk_h",
                                        name="wuk_h")
                        nc.scalar.dma_start(out=wuk_h[:], in_=wuk[h])
                        wuv_h = p2.tile([128, NKV, DV], bf16, tag="wuv_h",
                                        name="wuv_h")
                        nc.scalar.dma_start(out=wuv_h[:], in_=wuv[h])
                        wuv_hh.append(wuv_h)
                        wuk_hh.append(wuk_h)

                        # q_b transposed: nope [dn, tok] + rope [dr, tok]
                        # share one PSUM bank; only the first MM carries start
                        if hh == 0:
                            qnr_ps = pQ.tile([128, 512], f32, tag="qnr",
                                             name="qnr")
                        else:
                            qnr_ps = pD.tile([128, 512], f32, tag="den",
                                             name="qnrb")
                        for rc in range(NRC):
                            nc.tensor.matmul(
                                qnr_ps[:, 0:RPC], qbw[:, rc, 0:DN],
                                qcT[:, rc, :],
                                start=(rc == 0), stop=(rc == NRC - 1),
                                skip_group_check=True)
                            nc.tensor.matmul(
                                qnr_ps[0:DR, RPC:2 * RPC], qbw[:, rc, DN:QH],
                                qcT[:, rc, :],
                                start=False, stop=(rc == NRC - 1),
                                skip_group_check=True)
                        qnr_hh.append(qnr_ps)
                    # pass 2: rope + absorb chains for both heads
                    for hh in range(2):
                        qnr_ps = qnr_hh[hh]
                        wuk_h = wuk_hh[hh]
                        qnT = p2d.tile([128, RPC], bf16, tag="qnT", name="qnT")
                        nc.vector.tensor_copy(qnT[:], qnr_ps[:, 0:RPC])
                        # rope via signed pair-swap perm matmul: C*x + P(S*x)
                        tS = p2d.tile([64, RPC], bf16, tag="tS", name="tS")
                        nc.vector.tensor_mul(tS[:], qnr_ps[0:DR, RPC:2 * RPC],
                                             sinqT_sb[:])
                        tC = p2d.tile([64, RPC], f32, tag="tC", name="tC")
                        nc.vector.tensor_mul(tC[:], qnr_ps[0:DR, RPC:2 * RPC],
                                             cosqT_sb[:])
                        pr_ps = psp.tile([128, 2 * RPC], f32, tag="sp",
                                         name="prps")
                        nc.tensor.matmul(pr_ps[0:DR, 0:RPC], permT_sb[:], tS[:],
                                         start=True, stop=True)
                        nc.vector.tensor_add(qrT2[:, hh, :], tC[:],
                                             pr_ps[0:DR, 0:RPC])
                        # absorb w_uk into q
                        for rc in range(NKV):
                            lp = psp.tile([128, 2 * RPC], f32, tag="sp",
                                          name="lp")
                            nc.tensor.matmul(lp[:, 0:RPC], wuk_h[:, rc, :],
                                             qnT[:], start=True, stop=True)
                            nc.vector.tensor_copy(qT2[:, rc, hh, :],
                                                  lp[:, 0:RPC])

                    olT_ps = [pO.tile([128, 2, RPC], f32, tag="olT", name="olT")
                              for _ in range(NKV)]
                    den_ps = pD.tile([1, 2, RPC], f32, tag="den", name="den")
                    for kt in range(NTT):
                        q0 = 16 * kt
                        sp3 = psp.tile([128, 2, RPC], f32, tag="sp", name="sp")
                        for dc in range(NKV):
                            nc.tensor.matmul(
                                sp3[:, :, q0:],
                                kT_lat[:, dc, kt * 128:(kt + 1) * 128],
                                qT2[:, dc, :, q0:],
                                start=(dc == 0), stop=False,
                                skip_group_check=True)
                        nc.tensor.matmul(
                            sp3[:, :, q0:],
                            kT_rope[:, kt * 128:(kt + 1) * 128],
                            qrT2[:, :, q0:],
                            start=False, stop=True, skip_group_check=True)
                        eT = p2e.tile([128, 2, RPC], bf16, tag="eT", name="eT")
                        nc.scalar.activation(eT[:, :, q0:], sp3[:, :, q0:],
                                             AF.Exp)
                        for hh in range(2):
                            nc.vector.tensor_mul(eT[:, hh, q0:], eT[:, hh, q0:],
                                                 masks_sb[:, kt, q0:])
                        for rc in range(NKV):
                            nc.tensor.matmul(
                                olT_ps[rc][:, :, q0:],
                                c_hat[:, kt, rc * 128:(rc + 1) * 128],
                                eT[:, :, q0:],
                                start=(kt == 0), stop=(kt == NTT - 1),
                                skip_group_check=True)
                        nc.tensor.matmul(
                            den_ps[0:1, :, q0:], ones_sb[:], eT[:, :, q0:],
                            start=(kt == 0), stop=(kt == NTT - 1),
                            skip_group_check=True)
                    # normalize: reciprocal row, broadcast, scale o_v columns
                    den_sb = p2r.tile([1, 2, RPC], f32, tag="den_sb",
                                      name="den_sb")
                    nc.vector.tensor_copy(den_sb[:], den_ps[:])
                    rinv = p2r.tile([1, 2, RPC], f32, tag="rinv", name="rinv")
                    nc.vector.reciprocal(rinv[:], den_sb[:])
                    rb = p2r.tile([128, 2, RPC], f32, tag="rb", name="rb")
                    nc.gpsimd.partition_broadcast(rb[:], rinv[0:1, :, :])
                    olT_sb = p2d.tile([128, NKV, 2, RPC], bf16, tag="olT_sb",
                                      name="olT_sb")
                    for rc in range(NKV):
                        nc.vector.tensor_copy(olT_sb[:, rc, :, :],
                                              olT_ps[rc][:])
                    for hh in range(2):
                        ovp = psp.tile([128, 2 * RPC], f32, tag="sp", name="ovp")
                        for rc in range(NKV):
                            nc.tensor.matmul(
                                ovp[:, 0:RPC], wuv_hh[hh][:, rc, :],
                                olT_sb[:, rc, hh, :],
                                start=(rc == 0), stop=(rc == NKV - 1))
                        nc.vector.tensor_mul(o_vT[:, 2 * hp + hh, :],
                                             ovp[:, 0:RPC], rb[:, hh, :])


        # =================== phase 3: o_proj + residual + post-norm ===========
        if MAXPH >= 3:
            with tc.tile_pool(name="p3", bufs=3) as p3, \
                 tc.tile_pool(name="p3s", bufs=1) as p3s, \
                 tc.tile_pool(name="p3d", bufs=2) as p3d:
                hn = p3s.tile([128, NQT, H], bf16, name="hn")
                with tc.tile_pool(name="p3ps", bufs=2, space="PSUM") as p3ps:
                    op_ps = [p3ps.tile([128, H], f32, tag="opps", name="opps")
                             for _ in range(NQT)]
                    for hc in range(NH):
                        oww = p3.tile([128, H], bf16, tag="oww", name="oww")
                        nc.scalar.dma_start(out=oww[:], in_=ow_blk[hc])
                        for qt in range(NQT):
                            for nn in range(4):
                                nc.tensor.matmul(
                                    op_ps[qt][:, nn * 512:(nn + 1) * 512],
                                    o_vT[:, hc, qt * 128:(qt + 1) * 128],
                                    oww[:, nn * 512:(nn + 1) * 512],
                                    start=(hc == 0), stop=(hc == NH - 1))
                    ssq2 = p3s.tile([128, NQT], f32, name="ssq2")
                    for qt in range(NQT):
                        nc.vector.tensor_add(x_rows_sb[:, qt, :],
                                             x_rows_sb[:, qt, :], op_ps[qt][:])
                    for qt in range(NQT):
                        scr = p3d.tile([128, H], bf16, tag="scr3", name="scr3")
                        nc.vector.scalar_tensor_tensor(
                            scr[:], x_rows_sb[:, qt, :], 1.0, x_rows_sb[:, qt, :],
                            ALU.bypass, ALU.mult, accum_out=ssq2[:, qt:qt + 1])
                    nc.scalar.activation(ssq2[:], ssq2[:], AF.Ln, bias=eps_sb[:],
                                         scale=1.0 / H)
                    nc.scalar.activation(ssq2[:], ssq2[:], AF.Exp, scale=-0.5)
                    for qt in range(NQT):
                        nc.vector.tensor_scalar_mul(hn[:, qt, :],
                                                    x_rows_sb[:, qt, :],
                                                    ssq2[:, qt:qt + 1])
                for qt in range(NQT):
                    nc.sync.dma_start_transpose(
                        hnT[:, :, qt * 128:(qt + 1) * 128], hn[:, qt, :])


        # =================== phase 4: MLP ===================
        # dw tiles stream on the scalar-engine HWDGE ring (gw/uw own the sync
        # ring), with the first NPRE emitted inside the 4a loop so the down
        # weights prefetch while gate/up still computes.
        if MAXPH >= 4:
            NPRE = 6
            with tc.tile_pool(name="p4b", bufs=NPRE) as p4b, \
                 tc.tile_pool(name="p4s", bufs=2) as p4s:
                dw_tiles = {}

                def dw_fetch(it):
                    dw = p4b.tile([128, H], bf16, tag="dw", name="dw")
                    nc.scalar.dma_start(out=dw[:], in_=dw_blk[it])
                    dw_tiles[it] = dw

                with tc.tile_pool(name="p4", bufs=3) as p4, \
                     tc.tile_pool(name="p4ps", bufs=2, space="PSUM") as p4ps:
                    for it in range(NIT):
                        gw = p4.tile([128, NFC, 128], bf16, tag="gw", name="gw")
                        nc.sync.dma_start(out=gw[:], in_=gu_blk[0, it])
                        uw = p4.tile([128, NFC, 128], bf16, tag="uw", name="uw")
                        nc.sync.dma_start(out=uw[:], in_=gu_blk[1, it])
                        gp = p4ps.tile([128, RPC], f32, tag="gp", name="gp")
                        up = p4ps.tile([128, RPC], f32, tag="up", name="up")
                        for fc in range(NFC):
                            nc.tensor.matmul(gp[:], gw[:, fc, :], hnT[:, fc, :],
                                             start=(fc == 0),
                                             stop=(fc == NFC - 1))
                            nc.tensor.matmul(up[:], uw[:, fc, :], hnT[:, fc, :],
                                             start=(fc == 0),
                                             stop=(fc == NFC - 1))
                        gs = p4.tile([128, RPC], bf16, tag="gs", name="gs")
                        nc.scalar.activation(gs[:], gp[:], AF.Silu)
                        nc.vector.tensor_mul(act_all[:, it, :], gs[:], up[:])
                        if NIT - NPRE <= it < NIT:
                            dw_fetch(it - (NIT - NPRE))
                with tc.tile_pool(name="p4bps", bufs=2, space="PSUM") as p4bps:
                    o_ps = [p4bps.tile([128, H], f32, tag="ops", name="ops")
                            for _ in range(NQT)]
                    for it in range(NIT):
                        if it not in dw_tiles:
                            dw_fetch(it)
                        dw = dw_tiles.pop(it)
                        for qt in range(NQT):
                            for nn in range(4):
                                nc.tensor.matmul(
                                    o_ps[qt][:, nn * 512:(nn + 1) * 512],
                                    act_all[:, it, qt * 128:(qt + 1) * 128],
                                    dw[:, nn * 512:(nn + 1) * 512],
                                    start=(it == 0), stop=(it == NIT - 1))
                    for qt in range(NQT):
                        fin = p4s.tile([128, H], f32, tag="fin", name="fin")
                        nc.vector.tensor_add(fin[:], x_rows_sb[:, qt, :],
                                             o_ps[qt][:])
                        nc.sync.dma_start(out=out_rows[qt], in_=fin[:])

        if MAXPH < 4:
            with tc.tile_pool(name="pex", bufs=2) as pex:
                for qt in range(NQT):
                    fin = pex.tile([128, H], f32, tag="finx", name="finx")
                    nc.vector.tensor_copy(fin[:], x_rows_sb[:, qt, :])
                    nc.sync.dma_start(out=out_rows[qt], in_=fin[:])
    nc.compile()
    return nc


def _host_prep(inputs):
    f32 = np.float32
    bf = bfloat16
    x = np.asarray(inputs["hidden_states"], f32)
    pos = np.asarray(inputs["positions"]).astype(f32)

    lnw_in = np.asarray(inputs["input_ln_w"], f32)
    q_a_w = np.asarray(inputs["q_a_w"], f32) * lnw_in[:, None]
    kv_a_w = np.asarray(inputs["kv_a_w"], f32) * lnw_in[:, None]
    q_b_w = (np.asarray(inputs["q_b_w"], f32)
             * np.asarray(inputs["q_a_ln_w"], f32)[:, None]) * SCALE
    kvln = np.asarray(inputs["kv_a_ln_w"], f32)
    w_uk = np.asarray(inputs["w_uk"], f32) * kvln[:, None, None]
    w_uv = np.asarray(inputs["w_uv"], f32) * kvln[:, None, None]
    o_w = np.asarray(inputs["o_w"], f32)
    pln = np.asarray(inputs["post_ln_w"], f32)
    gate_w = np.asarray(inputs["gate_w"], f32) * pln[:, None]
    up_w = np.asarray(inputs["up_w"], f32) * pln[:, None]
    down_w = np.asarray(inputs["down_w"], f32)

    xT = np.ascontiguousarray(x.T)
    inv_freq = 1.0 / (THETA ** (np.arange(0, DR, 2, dtype=f32) / DR))
    ang = pos[:, None] * inv_freq
    cos_t = np.cos(ang).astype(f32)
    sin_t = np.sin(ang).astype(f32)

    gu = np.zeros((2, IPAD, H), f32)
    gu[0, :INTER] = gate_w.T
    gu[1, :INTER] = up_w.T

    rep = {
        "xstat": np.ascontiguousarray(x.reshape(NTT, 128, H).astype(bf)),
        "xT_blk": np.ascontiguousarray(
            xT.astype(bf).reshape(NFC, 128, NTT, 128).transpose(2, 1, 0, 3)),
        "qa_blk": np.ascontiguousarray(q_a_w.astype(bf).reshape(NFC, 128, QLR)),
        # qb_blk[h, rc, p, d] = q_b_w[rc*128+p, h*192+d]
        "qb_blk": np.ascontiguousarray(
            q_b_w.astype(bf).reshape(NRC, 128, NH, QH).transpose(2, 0, 1, 3)),
        "kva_blk": np.ascontiguousarray(
            kv_a_w.astype(bf).reshape(NFC, 128, KVLR + DR)),
        # wuk[h, d, rc, rr] = w_uk[rc*128+rr, h, d]
        "wuk": np.ascontiguousarray(
            w_uk.transpose(1, 2, 0).reshape(NH, 128, NKV, 128).astype(bf)),
        # wuv[h, p, rc, dv] = w_uv[rc*128+p, h, dv]
        "wuv": np.ascontiguousarray(
            w_uv.transpose(1, 0, 2).reshape(NH, NKV, 128, DV)
            .transpose(0, 2, 1, 3).astype(bf)),
        "ow_blk": np.ascontiguousarray(o_w.astype(bf).reshape(NH, 128, H)),
        "gu_blk": np.ascontiguousarray(
            gu.reshape(2, NIT, 128, NFC, 128).transpose(0, 1, 4, 3, 2)
            .astype(bf)),
        "dw_blk": np.ascontiguousarray(
            np.concatenate([down_w, np.zeros((IPAD - INTER, H), f32)], 0)
            .astype(bf).reshape(NIT, 128, H)),
        "cosk": np.ascontiguousarray(
            cos_t.reshape(NTT, 128, DR // 2).transpose(1, 0, 2)),
        "sink": np.ascontiguousarray(
            sin_t.reshape(NTT, 128, DR // 2).transpose(1, 0, 2)),
        "eye": np.eye(128, dtype=bf),
        "ones": np.ones((128, 1), bf),
    }
    # rope pair-swap permutation: out = M @ v; lhsT = M.T
    M = np.zeros((DR, DR), f32)
    for i in range(DR // 2):
        M[2 * i, 2 * i + 1] = -1.0
        M[2 * i + 1, 2 * i] = 1.0
    rep["permT"] = np.ascontiguousarray(M.T).astype(bf)

    per_core = []
    for c in range(NCORES):
        rows = np.arange(c, T, NCORES)
        m = dict(rep)
        m["x_rows"] = np.ascontiguousarray(x[rows].reshape(NQT, 128, H))
        m["xTc"] = np.ascontiguousarray(
            xT[:, rows].astype(bf).reshape(NFC, 128, RPC))
        # [64, RPC] rope tables, row d -> freq d//2
        m["cosqT"] = np.ascontiguousarray(
            np.repeat(cos_t[rows].T, 2, axis=0).astype(f32))
        m["sinqT"] = np.ascontiguousarray(
            np.repeat(sin_t[rows].T, 2, axis=0).astype(f32))
        mask = np.zeros((NTT, 128, RPC), f32)
        kpos = np.arange(128)
        for kt in range(NTT):
            gk = kt * 128 + kpos
            mask[kt] = (gk[:, None] <= rows[None, :]).astype(f32)
        m["masks"] = mask.astype(bf)
        per_core.append(m)
    return per_core


def kernel(**inputs):
    from concourse import bass_utils

    if "nc" not in _CACHE:
        _CACHE["nc"] = _build_module()
    nc = _CACHE["nc"]

    import os
    in_maps = _host_prep(inputs)
    trace = bool(os.environ.get("BASS_KERNEL_TRACE"))
    res = bass_utils.run_bass_kernel_spmd(nc, in_maps,
                                          core_ids=list(range(NCORES)),
                                          trace=trace)
    if trace and res.exec_time_ns is not None:
        print(f"HW exec time: {res.exec_time_ns} ns")
        _CACHE["last_result"] = res
    out = np.zeros((T, H), np.float32)
    for c in range(NCORES):
        rows = np.arange(c, T, NCORES)
        out[rows] = res.results[c]["out_rows"].reshape(RPC, H)
    return out

